# revision 21
# baseline (speedup 1.0000x reference)
"""Trainium2 Bass kernel for nn_MultiHeadAttention (B=4, S=2048, D=1024, H=16, DH=64).

Sharding: 8 cores = 4 batches x 2 query-halves. Each core computes, for its
(batch b, query half): Q/K/V projections, masked softmax attention over the
full key length, and the output projection, entirely on-device.

Device-side layout is fully transposed (feature-major) so every matmul has its
contraction on the partition dim:
  QT = Wq^T xqT / 8         [HDH, Sq]   (1/8 score scale + bq folded in)
  KT = Wk^T xkT             [HDH, S]    (spilled to DRAM, per-pair reload)
  V  = (Wv^T xvT)^T         [S, HDH]    stored interleaved per head as
                                        [s, h, 65] with a ones column, so the
                                        PV matmul's PSUM row 64 accumulates
                                        l = sum_k exp(scoresT) for free
  scoresT_h = KT_h^T QT_h   [S, Sq]     (per head; 2 heads packed in PE rows)
  expT = exp(scoresT + mask_bias[k])    (no row-max: |scores| <= ~4)
  outT_h = V_h^T expT / l   [DH, Sq]
  yT = Wo^T outT + bo'      [D, Sq]
Key-padding mask enters as a per-partition bias (0 / -40) on the Exp
activation. bk is dropped (softmax-invariant); bv,bo fold into bo' = bv@Wo+bo
host-side (exact, since softmax rows sum to 1).
"""

import os
import sys
import numpy as np

if "/opt/trn_rl_repo" not in sys.path:
    sys.path.insert(0, "/opt/trn_rl_repo")

import concourse.bass as bass
import concourse.mybir as mybir
import concourse.tile as tile
from concourse import bacc
from concourse.bass_utils import run_bass_kernel_spmd

B, S, D = 4, 2048, 1024
H, DH = 16, 64
HDH = H * DH                      # 1024
SQ = S // 2                       # 1024 queries per core
P = 128
DC = D // P                       # 8 contraction chunks
NJ = 8                            # head pairs (2 heads x 64 rows = 128)
KC = S // P                       # 16 key chunks
SC = S // P                       # 16 s chunks for V
VW = DH + 1                       # 65: V columns per head + ones column
F32 = mybir.dt.float32
F32R = mybir.dt.float32r
BF16 = mybir.dt.bfloat16
MASK_NEG = -40.0

_CACHE = {}


def build_bass():
    nc = bacc.Bacc("TRN2", target_bir_lowering=False, debug=False)

    xqT = nc.dram_tensor("xqT", [D, SQ], F32R, kind="ExternalInput").ap()
    xkT = nc.dram_tensor("xkT", [D, S], F32R, kind="ExternalInput").ap()
    xvT = nc.dram_tensor("xvT", [D, S], F32R, kind="ExternalInput").ap()
    wq = nc.dram_tensor("wq", [D, HDH], F32R, kind="ExternalInput").ap()
    wk = nc.dram_tensor("wk", [D, HDH], F32R, kind="ExternalInput").ap()
    wv = nc.dram_tensor("wv", [D, HDH], F32R, kind="ExternalInput").ap()
    wo = nc.dram_tensor("wo", [HDH, D], BF16, kind="ExternalInput").ap()
    bq8 = nc.dram_tensor("bq8", [P, NJ], F32, kind="ExternalInput").ap()
    bo2 = nc.dram_tensor("bo2", [P, DC], F32, kind="ExternalInput").ap()
    maskb = nc.dram_tensor("maskb", [P, KC], F32, kind="ExternalInput").ap()
    yT = nc.dram_tensor("yT", [D, SQ], F32, kind="ExternalOutput").ap()

    Exp = mybir.ActivationFunctionType.Exp
    AOp = mybir.AluOpType

    with tile.TileContext(nc) as tc:
        with (
            tc.tile_pool(name="const", bufs=1) as cpool,
            tc.tile_pool(name="vres", bufs=1) as vpool,
            tc.tile_pool(name="ktdram", bufs=1, space="DRAM") as ktd,
            tc.tile_pool(name="rdram", bufs=2, space="DRAM") as rdp,
        ):
            maskb_sb = cpool.tile([P, KC], F32)
            nc.sync.dma_start(out=maskb_sb, in_=maskb)
            bq8_sb = cpool.tile([P, NJ], F32)
            nc.sync.dma_start(out=bq8_sb, in_=bq8)
            bo2_sb = cpool.tile([P, DC], F32)
            nc.sync.dma_start(out=bo2_sb, in_=bo2)

            # V interleaved per head: [p, sc, h, 65]; col 64 of each head = 1.0
            v_sb = vpool.tile([P, SC, H, VW], BF16)
            kt_dram = [
                ktd.tile([P, S], F32R, tag=f"ktd{j}", name=f"ktd{j}")
                for j in range(NJ)
            ]

            # ---- phase V: V = (Wv^T xvT)^T, all pairs --------------------
            # x loaded in column blocks so compute starts after block 0 and
            # freed blocks let the next phase's loads overlap this compute.
            with (
                tc.tile_pool(name="xv", bufs=1) as xvp,
                tc.tile_pool(name="wvp", bufs=1) as wvp,
                tc.tile_pool(name="pv", bufs=2, space="PSUM") as pvp,
            ):
                wv_sb = wvp.tile([P, DC, HDH], F32R)
                wv_ch = wv.rearrange("(c p) n -> p c n", p=P)
                xv_cb = []
                xv_ch = xvT.rearrange("(c p) s -> p c s", p=P)
                for cb in range(4):
                    t = xvp.tile([P, DC, 512], F32R, tag=f"xv{cb}",
                                 name=f"xv{cb}")
                    xv_cb.append(t)
                nc.sync.dma_start(
                    out=xv_cb[0], in_=xv_ch[:, :, 0:512]
                )
                for kc in range(DC):
                    nc.scalar.dma_start(out=wv_sb[:, kc, :], in_=wv_ch[:, kc, :])
                for cb in range(1, 4):
                    nc.sync.dma_start(
                        out=xv_cb[cb], in_=xv_ch[:, :, cb * 512:(cb + 1) * 512]
                    )
                for sc in range(SC):
                    cb, scl = sc // 4, sc % 4
                    ps = pvp.tile([P, HDH], F32, tag="pv")
                    for kc in range(DC):
                        lhsT = xv_cb[cb][:, kc, scl * P:(scl + 1) * P]
                        for nh in range(2):
                            nc.tensor.matmul(
                                ps[:, nh * 512:(nh + 1) * 512],
                                lhsT,
                                wv_sb[:, kc, nh * 512:(nh + 1) * 512],
                                start=(kc == 0),
                                stop=(kc == DC - 1),
                            )
                    # strided copy into the interleaved layout
                    nc.vector.tensor_copy(
                        v_sb[:, sc, :, 0:DH],
                        ps.rearrange("p (h d) -> p h d", d=DH),
                    )
                    nc.vector.tensor_scalar(
                        v_sb[:, sc, :, DH:VW], v_sb[:, sc, :, 0:1],
                        0.0, 1.0, AOp.mult, AOp.add,
                    )

            # ---- phase K: KT -> DRAM, all pairs --------------------------
            with (
                tc.tile_pool(name="xk", bufs=1) as xkp,
                tc.tile_pool(name="wkp", bufs=1) as wkp,
                tc.tile_pool(name="ktst", bufs=3) as ktstp,
                tc.tile_pool(name="pk", bufs=2, space="PSUM") as pkp,
            ):
                xk_cb = []
                xk_ch = xkT.rearrange("(c p) s -> p c s", p=P)
                for cb in range(4):
                    t = xkp.tile([P, DC, 512], F32R, tag=f"xk{cb}",
                                 name=f"xk{cb}")
                    eng = nc.sync if cb % 2 == 0 else nc.scalar
                    eng.dma_start(
                        out=t, in_=xk_ch[:, :, cb * 512:(cb + 1) * 512]
                    )
                    xk_cb.append(t)
                wk_sb = wkp.tile([P, DC, HDH], F32R)
                wk_ch = wk.rearrange("(c p) n -> p c n", p=P)
                for kc in range(DC):
                    eng = nc.scalar if kc % 2 == 0 else nc.sync
                    eng.dma_start(out=wk_sb[:, kc, :], in_=wk_ch[:, kc, :])
                for half in range(2):
                    for j in range(NJ):
                        ps = pkp.tile([P, SQ], F32, tag="pk")
                        for kc in range(DC):
                            lhsT = wk_sb[:, kc, j * P:(j + 1) * P]
                            for nh in range(2):
                                nc.tensor.matmul(
                                    ps[:, nh * 512:(nh + 1) * 512],
                                    lhsT,
                                    xk_cb[2 * half + nh][:, kc, :],
                                    start=(kc == 0),
                                    stop=(kc == DC - 1),
                                )
                        st = ktstp.tile([P, SQ], F32R, tag="ktst")
                        nc.vector.tensor_copy(st, ps)
                        nc.gpsimd.dma_start(
                            out=kt_dram[j][:, half * SQ:(half + 1) * SQ],
                            in_=st,
                        )

            # ---- phase Q: QT resident, all pairs -------------------------
            with tc.tile_pool(name="qtres", bufs=1) as qtpool:
                qt = [
                    qtpool.tile([P, SQ], F32R, tag=f"qt{j}", name=f"qt{j}")
                    for j in range(NJ)
                ]
                with (
                    tc.tile_pool(name="xq", bufs=1) as xqp,
                    tc.tile_pool(name="wqp", bufs=1) as wqp,
                    tc.tile_pool(name="pq", bufs=2, space="PSUM") as pqp,
                ):
                    xq_cb = []
                    xq_ch = xqT.rearrange("(c p) s -> p c s", p=P)
                    for cb in range(2):
                        t = xqp.tile([P, DC, 512], F32R, tag=f"xq{cb}",
                                     name=f"xq{cb}")
                        eng = nc.sync if cb % 2 == 0 else nc.scalar
                        eng.dma_start(
                            out=t, in_=xq_ch[:, :, cb * 512:(cb + 1) * 512]
                        )
                        xq_cb.append(t)
                    wq_sb = wqp.tile([P, DC, HDH], F32R)
                    wq_ch = wq.rearrange("(c p) n -> p c n", p=P)
                    for kc in range(DC):
                        eng = nc.scalar if kc % 2 == 0 else nc.sync
                        eng.dma_start(out=wq_sb[:, kc, :], in_=wq_ch[:, kc, :])
                    for j in range(NJ):
                        ps = pqp.tile([P, SQ], F32, tag="pq")
                        for kc in range(DC):
                            lhsT = wq_sb[:, kc, j * P:(j + 1) * P]
                            for nh in range(2):
                                nc.tensor.matmul(
                                    ps[:, nh * 512:(nh + 1) * 512],
                                    lhsT,
                                    xq_cb[nh][:, kc, :],
                                    start=(kc == 0),
                                    stop=(kc == DC - 1),
                                )
                        nc.vector.tensor_scalar(
                            qt[j], ps, 0.125, bq8_sb[:, j:j + 1],
                            AOp.mult, AOp.add,
                        )

                # ---- attention -------------------------------------------
                with tc.tile_pool(name="otres", bufs=1) as otpool:
                    ot = [
                        otpool.tile([P, SQ], BF16, tag=f"ot{j}", name=f"ot{j}")
                        for j in range(NJ)
                    ]
                    with (
                        tc.tile_pool(name="ktsb", bufs=2) as ktp,
                        tc.tile_pool(name="expp", bufs=6) as expp,
                        tc.tile_pool(name="lbp", bufs=2) as lbp,
                        tc.tile_pool(name="ps_s", bufs=2, space="PSUM") as pss,
                        tc.tile_pool(name="ps_o", bufs=1, space="PSUM") as pso,
                    ):
                        kt_tiles = {}

                        def load_kt(jj):
                            t = ktp.tile([P, S], F32R, tag="kt", name="kt_sb")
                            nc.sync.dma_start(out=t, in_=kt_dram[jj][:])
                            kt_tiles[jj] = t

                        load_kt(0)
                        for j in range(NJ):
                            if j + 1 < NJ:
                                load_kt(j + 1)  # prefetch next pair's KT
                            kt_sb = kt_tiles.pop(j)
                            ps_oa = pso.tile([VW, SQ], F32, tag="oa")
                            ps_ob = pso.tile([VW, SQ], F32, tag="ob")
                            ets = {}

                            def scores_exp(kc, j=j, kt_sb=kt_sb, ets=ets):
                                for hh in (0, 1):
                                    ps_s = pss.tile(
                                        [P, SQ], F32, tag="s", name="ps_s"
                                    )
                                    lhsT = kt_sb[hh * 64:(hh + 1) * 64,
                                                 kc * P:(kc + 1) * P]
                                    for nh in range(2):
                                        nc.tensor.matmul(
                                            ps_s[:, nh * 512:(nh + 1) * 512],
                                            lhsT,
                                            qt[j][hh * 64:(hh + 1) * 64,
                                                  nh * 512:(nh + 1) * 512],
                                            tile_position=(hh * 64, 0),
                                        )
                                    et = expp.tile(
                                        [P, SQ], BF16, tag="e", name="et"
                                    )
                                    nc.scalar.activation(
                                        et, ps_s, Exp,
                                        bias=maskb_sb[:, kc:kc + 1], scale=1.0,
                                    )
                                    ets[(kc, hh)] = et

                            def pv(kc, j=j, ets=ets, ps_oa=ps_oa, ps_ob=ps_ob):
                                for hh, ps_o in ((0, ps_oa), (1, ps_ob)):
                                    vh = v_sb[:, kc, 2 * j + hh, :]  # [128,65]
                                    et = ets.pop((kc, hh))
                                    for nh in range(2):
                                        nc.tensor.matmul(
                                            ps_o[:, nh * 512:(nh + 1) * 512],
                                            vh,
                                            et[:, nh * 512:(nh + 1) * 512],
                                            start=(kc == 0),
                                            stop=(kc == KC - 1),
                                        )

                            scores_exp(0)
                            scores_exp(1)
                            for kc in range(2, KC):
                                scores_exp(kc)
                                pv(kc - 2)
                            pv(KC - 2)
                            pv(KC - 1)

                            # release PSUM fast: copy to SBUF, then normalize
                            cpA = lbp.tile([VW, SQ], F32, tag="cpA", bufs=2)
                            nc.vector.tensor_copy(cpA, ps_oa)
                            nc.vector.reciprocal(
                                cpA[DH:VW, :], cpA[DH:VW, :]
                            )
                            cpB = lbp.tile([64, SQ], BF16, tag="cpB", bufs=2)
                            nc.vector.tensor_copy(cpB, ps_ob[0:DH, :])
                            rrB = lbp.tile([VW, SQ], F32, tag="rrB", bufs=2)
                            nc.vector.reciprocal(
                                rrB[DH:VW, :], ps_ob[DH:VW, :]
                            )
                            L_sb = lbp.tile([P, SQ], F32, tag="L", bufs=1)
                            for hh, rsrc in ((0, cpA), (1, rrB)):
                                rd = rdp.tile(
                                    [1, SQ], F32, tag="rd", name="rd"
                                )
                                nc.sync.dma_start(out=rd, in_=rsrc[DH:VW, :])
                                rd_b = bass.AP(
                                    tensor=rd.tensor, offset=rd.offset,
                                    ap=[[0, 64], rd.ap[-1]],
                                )
                                nc.sync.dma_start(
                                    out=L_sb[hh * 64:(hh + 1) * 64, :],
                                    in_=rd_b,
                                )
                            nc.vector.tensor_mul(
                                ot[j][0:64, :], cpA[0:DH, :], L_sb[0:64, :]
                            )
                            nc.gpsimd.dma_start(
                                out=ot[j][64:128, :], in_=cpB
                            )
                            nc.vector.tensor_mul(
                                ot[j][64:128, :], ot[j][64:128, :],
                                L_sb[64:128, :],
                            )

                    # ---- output projection -------------------------------
                    with (
                        tc.tile_pool(name="wop", bufs=4) as wop,
                        tc.tile_pool(name="ytp", bufs=3) as ytp,
                        tc.tile_pool(name="py", bufs=2, space="PSUM") as pyp,
                    ):
                        yt_ch = yT.rearrange("(c p) s -> c p s", p=P)
                        for dc in range(DC):
                            ps = pyp.tile([P, SQ], F32, tag="py")
                            for j in range(NJ):
                                wo_t = wop.tile([P, P], BF16, tag="wo")
                                nc.scalar.dma_start(
                                    out=wo_t,
                                    in_=wo[j * P:(j + 1) * P,
                                           dc * P:(dc + 1) * P],
                                )
                                for nh in range(2):
                                    nc.tensor.matmul(
                                        ps[:, nh * 512:(nh + 1) * 512],
                                        wo_t,
                                        ot[j][:, nh * 512:(nh + 1) * 512],
                                        start=(j == 0),
                                        stop=(j == NJ - 1),
                                    )
                            yt_sb = ytp.tile([P, SQ], F32, tag="yt")
                            nc.vector.tensor_scalar(
                                yt_sb, ps, bo2_sb[:, dc:dc + 1], None, AOp.add
                            )
                            nc.gpsimd.dma_start(out=yt_ch[dc], in_=yt_sb)

    nc.compile()
    return nc


def kernel(x_Q, x_K, x_V, src_batch_lens, Wq, bq, Wk, bk, Wv, bv, Wo, bo):
    x_Q = np.asarray(x_Q, dtype=np.float32)
    x_K = np.asarray(x_K, dtype=np.float32)
    x_V = np.asarray(x_V, dtype=np.float32)
    lens = np.asarray(src_batch_lens)
    Wq = np.ascontiguousarray(np.asarray(Wq, dtype=np.float32))
    Wk = np.ascontiguousarray(np.asarray(Wk, dtype=np.float32))
    Wv = np.ascontiguousarray(np.asarray(Wv, dtype=np.float32))
    import ml_dtypes
    Wo = np.ascontiguousarray(np.asarray(Wo, dtype=np.float32).astype(ml_dtypes.bfloat16))
    bq = np.asarray(bq, dtype=np.float32)
    bv = np.asarray(bv, dtype=np.float32)
    bo = np.asarray(bo, dtype=np.float32)

    if "nc" not in _CACHE:
        _CACHE["nc"] = build_bass()
    nc = _CACHE["nc"]

    bo2_full = (bv @ Wo + bo).astype(np.float32)
    bo2 = np.ascontiguousarray(bo2_full.reshape(DC, P).T)
    bq8 = np.ascontiguousarray((bq / 8.0).reshape(NJ, P).T)

    in_maps = []
    for c in range(8):
        b, hh = c // 2, c % 2
        q0 = hh * SQ
        k_idx = np.arange(S)
        mvec = np.where(k_idx < int(lens[b]), 0.0, MASK_NEG).astype(np.float32)
        in_maps.append({
            "xqT": np.ascontiguousarray(x_Q[b, q0:q0 + SQ, :].T),
            "xkT": np.ascontiguousarray(x_K[b].T),
            "xvT": np.ascontiguousarray(x_V[b].T),
            "wq": Wq, "wk": Wk, "wv": Wv, "wo": Wo,
            "bq8": bq8, "bo2": bo2,
            "maskb": np.ascontiguousarray(mvec.reshape(KC, P).T),
        })

    res = run_bass_kernel_spmd(nc, in_maps, core_ids=list(range(8)))

    out = np.empty((B, S, D), dtype=np.float32)
    for c in range(8):
        b, hh = c // 2, c % 2
        q0 = hh * SQ
        out[b, q0:q0 + SQ, :] = res.results[c]["yT"].T
    return out


# revision 24
# speedup vs baseline: 1.1949x; 1.1949x over previous
"""Trainium2 Bass kernel for nn_MultiHeadAttention (B=4, S=2048, D=1024, H=16, DH=64).

Sharding: 8 cores = 4 batches x 2 query-halves. Each core computes, for its
(batch b, query half): Q/K/V projections, masked softmax attention over the
full key length, and the output projection, entirely on-device.

Device-side layout is fully transposed (feature-major) so every matmul has its
contraction on the partition dim:
  QT = Wq^T xqT / 8         [HDH, Sq]   (1/8 score scale + bq folded in)
  KT = Wk^T xkT             [HDH, S]    (spilled to DRAM, per-pair reload)
  V  = (Wv^T xvT)^T         [S, HDH]    stored interleaved per head as
                                        [s, h, 65] with a ones column, so the
                                        PV matmul's PSUM row 64 accumulates
                                        l = sum_k exp(scoresT) for free
  scoresT_h = KT_h^T QT_h   [S, Sq]     (per head; 2 heads packed in PE rows)
  expT = exp(scoresT + mask_bias[k])    (no row-max: |scores| <= ~4)
  outT_h = V_h^T expT / l   [DH, Sq]
  yT = Wo^T outT + bo'      [D, Sq]
Key-padding mask enters as a per-partition bias (0 / -40) on the Exp
activation. bk is dropped (softmax-invariant); bv,bo fold into bo' = bv@Wo+bo
host-side (exact, since softmax rows sum to 1).
"""

import os
import sys
import numpy as np

if "/opt/trn_rl_repo" not in sys.path:
    sys.path.insert(0, "/opt/trn_rl_repo")

import concourse.bass as bass
import concourse.mybir as mybir
import concourse.tile as tile
from concourse import bacc
from concourse.bass_utils import run_bass_kernel_spmd

B, S, D = 4, 2048, 1024
H, DH = 16, 64
HDH = H * DH                      # 1024
SQ = S // 2                       # 1024 queries per core
P = 128
DC = D // P                       # 8 contraction chunks
NJ = 8                            # head pairs (2 heads x 64 rows = 128)
KC = S // P                       # 16 key chunks
SC = S // P                       # 16 s chunks for V
VW = DH + 1                       # 65: V columns per head + ones column
F32 = mybir.dt.float32
F32R = mybir.dt.float32r
BF16 = mybir.dt.bfloat16
MASK_NEG = -40.0

_CACHE = {}


def build_bass(kc_lim=KC):
    nc = bacc.Bacc("TRN2", target_bir_lowering=False, debug=False)
    klen = kc_lim * P                # keys actually attended (rest fully masked)
    kpad = ((klen + 511) // 512) * 512   # KT buffers padded to 512-col blocks

    xqT = nc.dram_tensor("xqT", [D, SQ], F32R, kind="ExternalInput").ap()
    xkT = nc.dram_tensor("xkT", [D, S], F32R, kind="ExternalInput").ap()
    xvT = nc.dram_tensor("xvT", [D, S], F32R, kind="ExternalInput").ap()
    wq = nc.dram_tensor("wq", [D, HDH], F32R, kind="ExternalInput").ap()
    wk = nc.dram_tensor("wk", [D, HDH], F32R, kind="ExternalInput").ap()
    wv = nc.dram_tensor("wv", [D, HDH], F32R, kind="ExternalInput").ap()
    wo = nc.dram_tensor("wo", [HDH, D], F32R, kind="ExternalInput").ap()
    bq8 = nc.dram_tensor("bq8", [P, NJ], F32, kind="ExternalInput").ap()
    bo2 = nc.dram_tensor("bo2", [P, DC], F32, kind="ExternalInput").ap()
    maskb = nc.dram_tensor("maskb", [P, KC], F32, kind="ExternalInput").ap()
    yT = nc.dram_tensor("yT", [D, SQ], F32, kind="ExternalOutput").ap()

    Exp = mybir.ActivationFunctionType.Exp
    AOp = mybir.AluOpType

    with tile.TileContext(nc) as tc:
        with (
            tc.tile_pool(name="const", bufs=1) as cpool,
            tc.tile_pool(name="vres", bufs=1) as vpool,
            tc.tile_pool(name="ktdram", bufs=1, space="DRAM") as ktd,
            tc.tile_pool(name="rdram", bufs=2, space="DRAM") as rdp,
        ):
            maskb_sb = cpool.tile([P, KC], F32)
            nc.sync.dma_start(out=maskb_sb, in_=maskb)
            bq8_sb = cpool.tile([P, NJ], F32)
            nc.sync.dma_start(out=bq8_sb, in_=bq8)
            bo2_sb = cpool.tile([P, DC], F32)
            nc.sync.dma_start(out=bo2_sb, in_=bo2)

            # V interleaved per head: [p, sc, h, 65]; col 64 of each head = 1.0
            v_sb = vpool.tile([P, SC, H, VW], F32R)
            kt_dram = [
                ktd.tile([P, kpad], F32R, tag=f"ktd{j}", name=f"ktd{j}")
                for j in range(NJ)
            ]

            # ---- phase V: V = (Wv^T xvT)^T, all pairs --------------------
            # x loaded in column blocks so compute starts after block 0 and
            # freed blocks let the next phase's loads overlap this compute.
            with (
                tc.tile_pool(name="xv", bufs=1) as xvp,
                tc.tile_pool(name="wvp", bufs=1) as wvp,
                tc.tile_pool(name="pv", bufs=2, space="PSUM") as pvp,
            ):
                wv_sb = wvp.tile([P, DC, HDH], F32R)
                wv_ch = wv.rearrange("(c p) n -> p c n", p=P)
                xv_cb = []
                xv_ch = xvT.rearrange("(c p) s -> p c s", p=P)
                for cb in range(4):
                    t = xvp.tile([P, DC, 512], F32R, tag=f"xv{cb}",
                                 name=f"xv{cb}")
                    xv_cb.append(t)
                nc.sync.dma_start(
                    out=xv_cb[0], in_=xv_ch[:, :, 0:512]
                )
                for kc in range(DC):
                    nc.scalar.dma_start(out=wv_sb[:, kc, :], in_=wv_ch[:, kc, :])
                for cb in range(1, 4):
                    nc.sync.dma_start(
                        out=xv_cb[cb], in_=xv_ch[:, :, cb * 512:(cb + 1) * 512]
                    )
                for sc in range(kc_lim):
                    cb, scl = sc // 4, sc % 4
                    ps = pvp.tile([P, HDH], F32, tag="pv")
                    for kc in range(DC):
                        lhsT = xv_cb[cb][:, kc, scl * P:(scl + 1) * P]
                        for nh in range(2):
                            nc.tensor.matmul(
                                ps[:, nh * 512:(nh + 1) * 512],
                                lhsT,
                                wv_sb[:, kc, nh * 512:(nh + 1) * 512],
                                start=(kc == 0),
                                stop=(kc == DC - 1),
                            )
                    # strided copy into the interleaved layout
                    nc.vector.tensor_copy(
                        v_sb[:, sc, :, 0:DH],
                        ps.rearrange("p (h d) -> p h d", d=DH),
                    )
                    nc.vector.tensor_scalar(
                        v_sb[:, sc, :, DH:VW], v_sb[:, sc, :, 0:1],
                        0.0, 1.0, AOp.mult, AOp.add,
                    )

            # ---- phase K: KT -> DRAM, all pairs --------------------------
            with (
                tc.tile_pool(name="xk", bufs=1) as xkp,
                tc.tile_pool(name="wkp", bufs=1) as wkp,
                tc.tile_pool(name="ktst", bufs=3) as ktstp,
                tc.tile_pool(name="pk", bufs=2, space="PSUM") as pkp,
            ):
                xk_cb = []
                xk_ch = xkT.rearrange("(c p) s -> p c s", p=P)
                for cb in range(4):
                    t = xkp.tile([P, DC, 512], F32R, tag=f"xk{cb}",
                                 name=f"xk{cb}")
                    eng = nc.sync if cb % 2 == 0 else nc.scalar
                    eng.dma_start(
                        out=t, in_=xk_ch[:, :, cb * 512:(cb + 1) * 512]
                    )
                    xk_cb.append(t)
                wk_sb = wkp.tile([P, DC, HDH], F32R)
                wk_ch = wk.rearrange("(c p) n -> p c n", p=P)
                for kc in range(DC):
                    eng = nc.scalar if kc % 2 == 0 else nc.sync
                    eng.dma_start(out=wk_sb[:, kc, :], in_=wk_ch[:, kc, :])
                n_kb = kpad // 512   # 512-col blocks of KT kept
                for half in range(2):
                    blks = [b for b in (2 * half, 2 * half + 1) if b < n_kb]
                    if not blks:
                        continue
                    w_cols = 512 * len(blks)
                    for j in range(NJ):
                        ps = pkp.tile([P, SQ], F32, tag="pk")
                        for kc in range(DC):
                            lhsT = wk_sb[:, kc, j * P:(j + 1) * P]
                            for bi, b in enumerate(blks):
                                nc.tensor.matmul(
                                    ps[:, bi * 512:(bi + 1) * 512],
                                    lhsT,
                                    xk_cb[b][:, kc, :],
                                    start=(kc == 0),
                                    stop=(kc == DC - 1),
                                )
                        st = ktstp.tile([P, SQ], F32R, tag="ktst")
                        nc.vector.tensor_copy(st[:, 0:w_cols], ps[:, 0:w_cols])
                        nc.gpsimd.dma_start(
                            out=kt_dram[j][:, half * SQ:half * SQ + w_cols],
                            in_=st[:, 0:w_cols],
                        )

            # ---- phase Q: QT resident, all pairs -------------------------
            with tc.tile_pool(name="qtres", bufs=1) as qtpool:
                qt = [
                    qtpool.tile([P, SQ], F32R, tag=f"qt{j}", name=f"qt{j}")
                    for j in range(NJ)
                ]
                with (
                    tc.tile_pool(name="xq", bufs=1) as xqp,
                    tc.tile_pool(name="wqp", bufs=1) as wqp,
                    tc.tile_pool(name="pq", bufs=2, space="PSUM") as pqp,
                ):
                    xq_cb = []
                    xq_ch = xqT.rearrange("(c p) s -> p c s", p=P)
                    for cb in range(2):
                        t = xqp.tile([P, DC, 512], F32R, tag=f"xq{cb}",
                                     name=f"xq{cb}")
                        eng = nc.sync if cb % 2 == 0 else nc.scalar
                        eng.dma_start(
                            out=t, in_=xq_ch[:, :, cb * 512:(cb + 1) * 512]
                        )
                        xq_cb.append(t)
                    wq_sb = wqp.tile([P, DC, HDH], F32R)
                    wq_ch = wq.rearrange("(c p) n -> p c n", p=P)
                    for kc in range(DC):
                        eng = nc.scalar if kc % 2 == 0 else nc.sync
                        eng.dma_start(out=wq_sb[:, kc, :], in_=wq_ch[:, kc, :])
                    for j in range(NJ):
                        ps = pqp.tile([P, SQ], F32, tag="pq")
                        for kc in range(DC):
                            lhsT = wq_sb[:, kc, j * P:(j + 1) * P]
                            for nh in range(2):
                                nc.tensor.matmul(
                                    ps[:, nh * 512:(nh + 1) * 512],
                                    lhsT,
                                    xq_cb[nh][:, kc, :],
                                    start=(kc == 0),
                                    stop=(kc == DC - 1),
                                )
                        nc.vector.tensor_scalar(
                            qt[j], ps, 0.125, bq8_sb[:, j:j + 1],
                            AOp.mult, AOp.add,
                        )

                # ---- attention -------------------------------------------
                with tc.tile_pool(name="otres", bufs=1) as otpool:
                    ot = [
                        otpool.tile([P, SQ], F32R, tag=f"ot{j}", name=f"ot{j}")
                        for j in range(NJ)
                    ]
                    with (
                        tc.tile_pool(name="ktsb", bufs=2) as ktp,
                        tc.tile_pool(name="expp", bufs=3) as expp,
                        tc.tile_pool(name="lbp", bufs=2) as lbp,
                        tc.tile_pool(name="ps_s", bufs=1, space="PSUM") as pss,
                        tc.tile_pool(name="ps_o", bufs=1, space="PSUM") as pso,
                    ):
                        kt_tiles = {}

                        def load_kt(jj):
                            t = ktp.tile([P, kpad], F32R, tag="kt", name="kt_sb")
                            nc.sync.dma_start(out=t, in_=kt_dram[jj][:])
                            kt_tiles[jj] = t

                        load_kt(0)
                        for j in range(NJ):
                            if j + 1 < NJ:
                                load_kt(j + 1)  # prefetch next pair's KT
                            kt_sb = kt_tiles.pop(j)
                            ps_oa = pso.tile([VW, SQ], F32, tag="oa")
                            ps_ob = pso.tile([VW, SQ], F32, tag="ob")
                            ets = {}

                            def scores_exp(kc, j=j, kt_sb=kt_sb, ets=ets):
                                ps_s = pss.tile(
                                    [P, 2 * SQ], F32, tag="s", name="ps_s"
                                )
                                for hh in (0, 1):
                                    lhsT = kt_sb[hh * 64:(hh + 1) * 64,
                                                 kc * P:(kc + 1) * P]
                                    for nh in range(2):
                                        nc.tensor.matmul(
                                            ps_s[:, hh * SQ + nh * 512:
                                                 hh * SQ + (nh + 1) * 512],
                                            lhsT,
                                            qt[j][hh * 64:(hh + 1) * 64,
                                                  nh * 512:(nh + 1) * 512],
                                            tile_position=(hh * 64, 0),
                                        )
                                et = expp.tile(
                                    [P, 2 * SQ], F32R, tag="e", name="et"
                                )
                                nc.scalar.activation(
                                    et, ps_s, Exp,
                                    bias=maskb_sb[:, kc:kc + 1], scale=1.0,
                                )
                                ets[kc] = et

                            def pv(kc, j=j, ets=ets, ps_oa=ps_oa, ps_ob=ps_ob):
                                et = ets.pop(kc)
                                for hh, ps_o in ((0, ps_oa), (1, ps_ob)):
                                    vh = v_sb[:, kc, 2 * j + hh, :]  # [128,65]
                                    for nh in range(2):
                                        nc.tensor.matmul(
                                            ps_o[:, nh * 512:(nh + 1) * 512],
                                            vh,
                                            et[:, hh * SQ + nh * 512:
                                               hh * SQ + (nh + 1) * 512],
                                            start=(kc == 0),
                                            stop=(kc == kc_lim - 1),
                                        )

                            scores_exp(0)
                            scores_exp(1)
                            for kc in range(2, kc_lim):
                                scores_exp(kc)
                                pv(kc - 2)
                            pv(kc_lim - 2)
                            pv(kc_lim - 1)

                            # release PSUM fast: copy to SBUF, then normalize
                            cpA = lbp.tile([VW, SQ], F32, tag="cpA", bufs=2)
                            nc.vector.tensor_copy(cpA, ps_oa)
                            nc.vector.reciprocal(
                                cpA[DH:VW, :], cpA[DH:VW, :]
                            )
                            cpB = lbp.tile([64, SQ], F32R, tag="cpB", bufs=2)
                            nc.vector.tensor_copy(cpB, ps_ob[0:DH, :])
                            rrB = lbp.tile([VW, SQ], F32, tag="rrB", bufs=2)
                            nc.vector.reciprocal(
                                rrB[DH:VW, :], ps_ob[DH:VW, :]
                            )
                            L_sb = lbp.tile([P, SQ], F32, tag="L", bufs=1)
                            for hh, rsrc in ((0, cpA), (1, rrB)):
                                rd = rdp.tile(
                                    [1, SQ], F32, tag="rd", name="rd"
                                )
                                nc.sync.dma_start(out=rd, in_=rsrc[DH:VW, :])
                                rd_b = bass.AP(
                                    tensor=rd.tensor, offset=rd.offset,
                                    ap=[[0, 64], rd.ap[-1]],
                                )
                                nc.sync.dma_start(
                                    out=L_sb[hh * 64:(hh + 1) * 64, :],
                                    in_=rd_b,
                                )
                            nc.vector.tensor_mul(
                                ot[j][0:64, :], cpA[0:DH, :], L_sb[0:64, :]
                            )
                            nc.gpsimd.dma_start(
                                out=ot[j][64:128, :], in_=cpB
                            )
                            nc.vector.tensor_mul(
                                ot[j][64:128, :], ot[j][64:128, :],
                                L_sb[64:128, :],
                            )

                    # ---- output projection -------------------------------
                    with (
                        tc.tile_pool(name="wop", bufs=4) as wop,
                        tc.tile_pool(name="ytp", bufs=3) as ytp,
                        tc.tile_pool(name="py", bufs=2, space="PSUM") as pyp,
                    ):
                        yt_ch = yT.rearrange("(c p) s -> c p s", p=P)
                        for dc in range(DC):
                            ps = pyp.tile([P, SQ], F32, tag="py")
                            for j in range(NJ):
                                wo_t = wop.tile([P, P], F32R, tag="wo")
                                nc.scalar.dma_start(
                                    out=wo_t,
                                    in_=wo[j * P:(j + 1) * P,
                                           dc * P:(dc + 1) * P],
                                )
                                for nh in range(2):
                                    nc.tensor.matmul(
                                        ps[:, nh * 512:(nh + 1) * 512],
                                        wo_t,
                                        ot[j][:, nh * 512:(nh + 1) * 512],
                                        start=(j == 0),
                                        stop=(j == NJ - 1),
                                    )
                            yt_sb = ytp.tile([P, SQ], F32, tag="yt")
                            nc.vector.tensor_scalar(
                                yt_sb, ps, bo2_sb[:, dc:dc + 1], None, AOp.add
                            )
                            nc.gpsimd.dma_start(out=yt_ch[dc], in_=yt_sb)

    nc.compile()
    return nc


def kernel(x_Q, x_K, x_V, src_batch_lens, Wq, bq, Wk, bk, Wv, bv, Wo, bo):
    x_Q = np.asarray(x_Q, dtype=np.float32)
    x_K = np.asarray(x_K, dtype=np.float32)
    x_V = np.asarray(x_V, dtype=np.float32)
    lens = np.asarray(src_batch_lens)
    Wq = np.ascontiguousarray(np.asarray(Wq, dtype=np.float32))
    Wk = np.ascontiguousarray(np.asarray(Wk, dtype=np.float32))
    Wv = np.ascontiguousarray(np.asarray(Wv, dtype=np.float32))
    Wo = np.ascontiguousarray(np.asarray(Wo, dtype=np.float32))
    bq = np.asarray(bq, dtype=np.float32)
    bv = np.asarray(bv, dtype=np.float32)
    bo = np.asarray(bo, dtype=np.float32)

    maxlen = int(np.max(lens))
    maxlen = max(1, min(S, maxlen))
    kc_lim = (maxlen + P - 1) // P
    if kc_lim not in _CACHE:
        _CACHE[kc_lim] = build_bass(kc_lim)
    nc = _CACHE[kc_lim]

    bo2_full = (bv @ Wo + bo).astype(np.float32)
    bo2 = np.ascontiguousarray(bo2_full.reshape(DC, P).T)
    bq8 = np.ascontiguousarray((bq / 8.0).reshape(NJ, P).T)

    in_maps = []
    for c in range(8):
        b, hh = c // 2, c % 2
        q0 = hh * SQ
        k_idx = np.arange(S)
        mvec = np.where(k_idx < int(lens[b]), 0.0, MASK_NEG).astype(np.float32)
        in_maps.append({
            "xqT": np.ascontiguousarray(x_Q[b, q0:q0 + SQ, :].T),
            "xkT": np.ascontiguousarray(x_K[b].T),
            "xvT": np.ascontiguousarray(x_V[b].T),
            "wq": Wq, "wk": Wk, "wv": Wv, "wo": Wo,
            "bq8": bq8, "bo2": bo2,
            "maskb": np.ascontiguousarray(mvec.reshape(KC, P).T),
        })

    res = run_bass_kernel_spmd(nc, in_maps, core_ids=list(range(8)))

    out = np.empty((B, S, D), dtype=np.float32)
    for c in range(8):
        b, hh = c // 2, c % 2
        q0 = hh * SQ
        out[b, q0:q0 + SQ, :] = res.results[c]["yT"].T
    return out


# revision 26
# speedup vs baseline: 1.3023x; 1.0899x over previous
"""Trainium2 Bass kernel for nn_MultiHeadAttention (B=4, S=2048, D=1024, H=16, DH=64).

Sharding: 8 cores = 4 batches x 2 query-halves. Each core computes, for its
(batch b, query half): Q/K/V projections, masked softmax attention over the
full key length, and the output projection, entirely on-device.

Device-side layout is fully transposed (feature-major) so every matmul has its
contraction on the partition dim:
  QT = Wq^T xqT / 8         [HDH, Sq]   (1/8 score scale + bq folded in)
  KT = Wk^T xkT             [HDH, S]    (spilled to DRAM, per-pair reload)
  V  = (Wv^T xvT)^T         [S, HDH]    stored interleaved per head as
                                        [s, h, 65] with a ones column, so the
                                        PV matmul's PSUM row 64 accumulates
                                        l = sum_k exp(scoresT) for free
  scoresT_h = KT_h^T QT_h   [S, Sq]     (per head; 2 heads packed in PE rows)
  expT = exp(scoresT + mask_bias[k])    (no row-max: |scores| <= ~4)
  outT_h = V_h^T expT / l   [DH, Sq]
  yT = Wo^T outT + bo'      [D, Sq]
Key-padding mask enters as a per-partition bias (0 / -40) on the Exp
activation. bk is dropped (softmax-invariant); bv,bo fold into bo' = bv@Wo+bo
host-side (exact, since softmax rows sum to 1).
"""

import os
import sys
import numpy as np

if "/opt/trn_rl_repo" not in sys.path:
    sys.path.insert(0, "/opt/trn_rl_repo")

import concourse.bass as bass
import concourse.mybir as mybir
import concourse.tile as tile
from concourse import bacc
from concourse.bass_utils import run_bass_kernel_spmd

B, S, D = 4, 2048, 1024
H, DH = 16, 64
HDH = H * DH                      # 1024
SQ = S // 2                       # 1024 queries per core
P = 128
DC = D // P                       # 8 contraction chunks
NJ = 8                            # head pairs (2 heads x 64 rows = 128)
KC = S // P                       # 16 key chunks
SC = S // P                       # 16 s chunks for V
VW = DH + 1                       # 65: V columns per head + ones column
F32 = mybir.dt.float32
F32R = mybir.dt.float32r
BF16 = mybir.dt.bfloat16
MASK_NEG = -40.0

_CACHE = {}


def build_bass(kc_lim=KC):
    nc = bacc.Bacc("TRN2", target_bir_lowering=False, debug=False)
    klen = kc_lim * P                # keys actually attended (rest fully masked)
    kpad = ((klen + 511) // 512) * 512   # KT buffers padded to 512-col blocks

    xqT = nc.dram_tensor("xqT", [D, SQ], F32R, kind="ExternalInput").ap()
    xkT = nc.dram_tensor("xkT", [D, S], F32R, kind="ExternalInput").ap()
    xvT = nc.dram_tensor("xvT", [D, S], F32R, kind="ExternalInput").ap()
    wq = nc.dram_tensor("wq", [D, HDH], F32R, kind="ExternalInput").ap()
    wk = nc.dram_tensor("wk", [D, HDH], F32R, kind="ExternalInput").ap()
    wv = nc.dram_tensor("wv", [D, HDH], F32R, kind="ExternalInput").ap()
    wo = nc.dram_tensor("wo", [HDH, D], F32R, kind="ExternalInput").ap()
    bq8 = nc.dram_tensor("bq8", [P, NJ], F32, kind="ExternalInput").ap()
    bo2 = nc.dram_tensor("bo2", [P, DC], F32, kind="ExternalInput").ap()
    maskb = nc.dram_tensor("maskb", [P, KC], F32, kind="ExternalInput").ap()
    yT = nc.dram_tensor("yT", [D, SQ], F32, kind="ExternalOutput").ap()

    Exp = mybir.ActivationFunctionType.Exp
    AOp = mybir.AluOpType

    with tile.TileContext(nc) as tc:
        with (
            tc.tile_pool(name="const", bufs=1) as cpool,
            tc.tile_pool(name="vres", bufs=1) as vpool,
            tc.tile_pool(name="ktdram", bufs=1, space="DRAM") as ktd,
            tc.tile_pool(name="rdram", bufs=2, space="DRAM") as rdp,
        ):
            maskb_sb = cpool.tile([P, KC], F32)
            nc.sync.dma_start(out=maskb_sb, in_=maskb)
            bq8_sb = cpool.tile([P, NJ], F32)
            nc.sync.dma_start(out=bq8_sb, in_=bq8)
            bo2_sb = cpool.tile([P, DC], F32)
            nc.sync.dma_start(out=bo2_sb, in_=bo2)

            # V interleaved per head: [p, sc, h, 65]; col 64 of each head = 1.0
            v_sb = vpool.tile([P, SC, H, VW], F32R)
            kt_dram = [
                ktd.tile([P, kpad], F32R, tag=f"ktd{j}", name=f"ktd{j}")
                for j in range(NJ)
            ]

            # ---- phase V: V = (Wv^T xvT)^T, all pairs --------------------
            # x loaded in column blocks so compute starts after block 0 and
            # freed blocks let the next phase's loads overlap this compute.
            with (
                tc.tile_pool(name="xv", bufs=1) as xvp,
                tc.tile_pool(name="wvp", bufs=1) as wvp,
                tc.tile_pool(name="pv", bufs=2, space="PSUM") as pvp,
            ):
                wv_sb = wvp.tile([P, DC, HDH], F32R)
                wv_ch = wv.rearrange("(c p) n -> p c n", p=P)
                xv_cb = []
                xv_ch = xvT.rearrange("(c p) s -> p c s", p=P)
                for cb in range(4):
                    t = xvp.tile([P, DC, 512], F32R, tag=f"xv{cb}",
                                 name=f"xv{cb}")
                    xv_cb.append(t)
                nc.sync.dma_start(
                    out=xv_cb[0], in_=xv_ch[:, :, 0:512]
                )
                for kc in range(DC):
                    nc.scalar.dma_start(out=wv_sb[:, kc, :], in_=wv_ch[:, kc, :])
                for cb in range(1, 4):
                    nc.sync.dma_start(
                        out=xv_cb[cb], in_=xv_ch[:, :, cb * 512:(cb + 1) * 512]
                    )
                for sc in range(kc_lim):
                    cb, scl = sc // 4, sc % 4
                    ps = pvp.tile([P, HDH], F32, tag="pv")
                    for kc in range(DC):
                        lhsT = xv_cb[cb][:, kc, scl * P:(scl + 1) * P]
                        for nh in range(2):
                            nc.tensor.matmul(
                                ps[:, nh * 512:(nh + 1) * 512],
                                lhsT,
                                wv_sb[:, kc, nh * 512:(nh + 1) * 512],
                                start=(kc == 0),
                                stop=(kc == DC - 1),
                            )
                    # strided copy into the interleaved layout
                    nc.vector.tensor_copy(
                        v_sb[:, sc, :, 0:DH],
                        ps.rearrange("p (h d) -> p h d", d=DH),
                    )
                    nc.vector.tensor_scalar(
                        v_sb[:, sc, :, DH:VW], v_sb[:, sc, :, 0:1],
                        0.0, 1.0, AOp.mult, AOp.add,
                    )

            # ---- phase K: KT -> DRAM, all pairs --------------------------
            with (
                tc.tile_pool(name="xk", bufs=1) as xkp,
                tc.tile_pool(name="wkp", bufs=1) as wkp,
                tc.tile_pool(name="ktst", bufs=3) as ktstp,
                tc.tile_pool(name="pk", bufs=2, space="PSUM") as pkp,
            ):
                xk_cb = []
                xk_ch = xkT.rearrange("(c p) s -> p c s", p=P)
                for cb in range(4):
                    t = xkp.tile([P, DC, 512], F32R, tag=f"xk{cb}",
                                 name=f"xk{cb}")
                    eng = nc.sync if cb % 2 == 0 else nc.scalar
                    eng.dma_start(
                        out=t, in_=xk_ch[:, :, cb * 512:(cb + 1) * 512]
                    )
                    xk_cb.append(t)
                wk_sb = wkp.tile([P, DC, HDH], F32R)
                wk_ch = wk.rearrange("(c p) n -> p c n", p=P)
                for kc in range(DC):
                    eng = nc.scalar if kc % 2 == 0 else nc.sync
                    eng.dma_start(out=wk_sb[:, kc, :], in_=wk_ch[:, kc, :])
                n_kb = kpad // 512   # 512-col blocks of KT kept
                for half in range(2):
                    blks = [b for b in (2 * half, 2 * half + 1) if b < n_kb]
                    if not blks:
                        continue
                    w_cols = 512 * len(blks)
                    for j in range(NJ):
                        ps = pkp.tile([P, SQ], F32, tag="pk")
                        for kc in range(DC):
                            lhsT = wk_sb[:, kc, j * P:(j + 1) * P]
                            for bi, b in enumerate(blks):
                                nc.tensor.matmul(
                                    ps[:, bi * 512:(bi + 1) * 512],
                                    lhsT,
                                    xk_cb[b][:, kc, :],
                                    start=(kc == 0),
                                    stop=(kc == DC - 1),
                                )
                        st = ktstp.tile([P, SQ], F32R, tag="ktst")
                        nc.vector.tensor_copy(st[:, 0:w_cols], ps[:, 0:w_cols])
                        nc.gpsimd.dma_start(
                            out=kt_dram[j][:, half * SQ:half * SQ + w_cols],
                            in_=st[:, 0:w_cols],
                        )

            # ---- phase Q: QT resident, all pairs -------------------------
            with tc.tile_pool(name="qtres", bufs=1) as qtpool:
                qt = [
                    qtpool.tile([P, SQ], F32R, tag=f"qt{j}", name=f"qt{j}")
                    for j in range(NJ)
                ]
                with (
                    tc.tile_pool(name="xq", bufs=1) as xqp,
                    tc.tile_pool(name="wqp", bufs=1) as wqp,
                    tc.tile_pool(name="pq", bufs=2, space="PSUM") as pqp,
                ):
                    xq_cb = []
                    xq_ch = xqT.rearrange("(c p) s -> p c s", p=P)
                    for cb in range(2):
                        t = xqp.tile([P, DC, 512], F32R, tag=f"xq{cb}",
                                     name=f"xq{cb}")
                        eng = nc.sync if cb % 2 == 0 else nc.scalar
                        eng.dma_start(
                            out=t, in_=xq_ch[:, :, cb * 512:(cb + 1) * 512]
                        )
                        xq_cb.append(t)
                    wq_sb = wqp.tile([P, DC, HDH], F32R)
                    wq_ch = wq.rearrange("(c p) n -> p c n", p=P)
                    for kc in range(DC):
                        eng = nc.scalar if kc % 2 == 0 else nc.sync
                        eng.dma_start(out=wq_sb[:, kc, :], in_=wq_ch[:, kc, :])
                    for j in range(NJ):
                        ps = pqp.tile([P, SQ], F32, tag="pq")
                        for kc in range(DC):
                            lhsT = wq_sb[:, kc, j * P:(j + 1) * P]
                            for nh in range(2):
                                nc.tensor.matmul(
                                    ps[:, nh * 512:(nh + 1) * 512],
                                    lhsT,
                                    xq_cb[nh][:, kc, :],
                                    start=(kc == 0),
                                    stop=(kc == DC - 1),
                                )
                        nc.vector.tensor_scalar(
                            qt[j], ps, 0.125, bq8_sb[:, j:j + 1],
                            AOp.mult, AOp.add,
                        )

                # ---- attention -------------------------------------------
                with tc.tile_pool(name="otres", bufs=1) as otpool:
                    ot = [
                        otpool.tile([P, SQ], F32R, tag=f"ot{j}", name=f"ot{j}")
                        for j in range(NJ)
                    ]
                    with (
                        tc.tile_pool(name="ktsb", bufs=2) as ktp,
                        tc.tile_pool(name="expp", bufs=3) as expp,
                        tc.tile_pool(name="lbp", bufs=2) as lbp,
                        tc.tile_pool(name="ps_s", bufs=1, space="PSUM") as pss,
                        tc.tile_pool(name="ps_o", bufs=1, space="PSUM") as pso,
                    ):
                        kt_tiles = {}

                        def load_kt(jj):
                            t = ktp.tile([P, kpad], F32R, tag="kt", name="kt_sb")
                            nc.sync.dma_start(out=t, in_=kt_dram[jj][:])
                            kt_tiles[jj] = t

                        load_kt(0)
                        for j in range(NJ):
                            if j + 1 < NJ:
                                load_kt(j + 1)  # prefetch next pair's KT
                            kt_sb = kt_tiles.pop(j)
                            ps_oa = pso.tile([VW, SQ], F32, tag="oa")
                            ps_ob = pso.tile([VW, SQ], F32, tag="ob")
                            ets = {}

                            def scores_exp(kc, j=j, kt_sb=kt_sb, ets=ets):
                                ps_s = pss.tile(
                                    [P, 2 * SQ], F32, tag="s", name="ps_s"
                                )
                                for hh in (0, 1):
                                    lhsT = kt_sb[hh * 64:(hh + 1) * 64,
                                                 kc * P:(kc + 1) * P]
                                    for nh in range(2):
                                        nc.tensor.matmul(
                                            ps_s[:, hh * SQ + nh * 512:
                                                 hh * SQ + (nh + 1) * 512],
                                            lhsT,
                                            qt[j][hh * 64:(hh + 1) * 64,
                                                  nh * 512:(nh + 1) * 512],
                                            tile_position=(hh * 64, 0),
                                        )
                                et = expp.tile(
                                    [P, 2 * SQ], F32R, tag="e", name="et"
                                )
                                nc.scalar.activation(
                                    et, ps_s, Exp,
                                    bias=maskb_sb[:, kc:kc + 1], scale=1.0,
                                )
                                ets[kc] = et

                            def pv(kc, j=j, ets=ets, ps_oa=ps_oa, ps_ob=ps_ob):
                                et = ets.pop(kc)
                                for hh, ps_o in ((0, ps_oa), (1, ps_ob)):
                                    vh = v_sb[:, kc, 2 * j + hh, :]  # [128,65]
                                    for nh in range(2):
                                        nc.tensor.matmul(
                                            ps_o[:, nh * 512:(nh + 1) * 512],
                                            vh,
                                            et[:, hh * SQ + nh * 512:
                                               hh * SQ + (nh + 1) * 512],
                                            start=(kc == 0),
                                            stop=(kc == kc_lim - 1),
                                        )

                            scores_exp(0)
                            scores_exp(1)
                            for kc in range(2, kc_lim):
                                scores_exp(kc)
                                pv(kc - 2)
                            pv(kc_lim - 2)
                            pv(kc_lim - 1)

                            # release PSUM fast: copy both heads to SBUF
                            # (incl. l rows); broadcast l, recip, then scale
                            cpA = lbp.tile([VW, SQ], F32, tag="cpA", bufs=2)
                            nc.vector.tensor_copy(cpA, ps_oa)
                            cpB = lbp.tile([VW, SQ], F32R, tag="cpB", bufs=2)
                            nc.vector.tensor_copy(cpB, ps_ob)
                            L_sb = lbp.tile([P, SQ], F32, tag="L", bufs=2)
                            for hh, rsrc in ((0, cpA), (1, cpB)):
                                rd = rdp.tile(
                                    [1, SQ], F32, tag="rd", name="rd"
                                )
                                nc.sync.dma_start(
                                    out=rd, in_=rsrc[DH:VW, :].bitcast(F32)
                                )
                                rd_b = bass.AP(
                                    tensor=rd.tensor, offset=rd.offset,
                                    ap=[[0, 64], rd.ap[-1]],
                                )
                                nc.sync.dma_start(
                                    out=L_sb[hh * 64:(hh + 1) * 64, :],
                                    in_=rd_b,
                                )
                            nc.vector.reciprocal_approx_fast(L_sb, L_sb)
                            nc.vector.tensor_mul(
                                ot[j][0:64, :], cpA[0:DH, :], L_sb[0:64, :]
                            )
                            nc.gpsimd.dma_start(
                                out=ot[j][64:128, :], in_=cpB[0:DH, :]
                            )
                            nc.vector.tensor_mul(
                                ot[j][64:128, :], ot[j][64:128, :],
                                L_sb[64:128, :],
                            )

                    # ---- output projection -------------------------------
                    with (
                        tc.tile_pool(name="wop", bufs=4) as wop,
                        tc.tile_pool(name="ytp", bufs=3) as ytp,
                        tc.tile_pool(name="py", bufs=2, space="PSUM") as pyp,
                    ):
                        yt_ch = yT.rearrange("(c p) s -> c p s", p=P)
                        for dc in range(DC):
                            ps = pyp.tile([P, SQ], F32, tag="py")
                            for j in range(NJ):
                                wo_t = wop.tile([P, P], F32R, tag="wo")
                                nc.scalar.dma_start(
                                    out=wo_t,
                                    in_=wo[j * P:(j + 1) * P,
                                           dc * P:(dc + 1) * P],
                                )
                                for nh in range(2):
                                    nc.tensor.matmul(
                                        ps[:, nh * 512:(nh + 1) * 512],
                                        wo_t,
                                        ot[j][:, nh * 512:(nh + 1) * 512],
                                        start=(j == 0),
                                        stop=(j == NJ - 1),
                                    )
                            yt_sb = ytp.tile([P, SQ], F32, tag="yt")
                            nc.vector.tensor_scalar(
                                yt_sb, ps, bo2_sb[:, dc:dc + 1], None, AOp.add
                            )
                            nc.gpsimd.dma_start(out=yt_ch[dc], in_=yt_sb)

    nc.compile()
    return nc


def kernel(x_Q, x_K, x_V, src_batch_lens, Wq, bq, Wk, bk, Wv, bv, Wo, bo):
    x_Q = np.asarray(x_Q, dtype=np.float32)
    x_K = np.asarray(x_K, dtype=np.float32)
    x_V = np.asarray(x_V, dtype=np.float32)
    lens = np.asarray(src_batch_lens)
    Wq = np.ascontiguousarray(np.asarray(Wq, dtype=np.float32))
    Wk = np.ascontiguousarray(np.asarray(Wk, dtype=np.float32))
    Wv = np.ascontiguousarray(np.asarray(Wv, dtype=np.float32))
    Wo = np.ascontiguousarray(np.asarray(Wo, dtype=np.float32))
    bq = np.asarray(bq, dtype=np.float32)
    bv = np.asarray(bv, dtype=np.float32)
    bo = np.asarray(bo, dtype=np.float32)

    maxlen = int(np.max(lens))
    maxlen = max(1, min(S, maxlen))
    kc_lim = (maxlen + P - 1) // P
    if kc_lim not in _CACHE:
        _CACHE[kc_lim] = build_bass(kc_lim)
    nc = _CACHE[kc_lim]

    bo2_full = (bv @ Wo + bo).astype(np.float32)
    bo2 = np.ascontiguousarray(bo2_full.reshape(DC, P).T)
    bq8 = np.ascontiguousarray((bq / 8.0).reshape(NJ, P).T)

    in_maps = []
    for c in range(8):
        b, hh = c // 2, c % 2
        q0 = hh * SQ
        k_idx = np.arange(S)
        mvec = np.where(k_idx < int(lens[b]), 0.0, MASK_NEG).astype(np.float32)
        in_maps.append({
            "xqT": np.ascontiguousarray(x_Q[b, q0:q0 + SQ, :].T),
            "xkT": np.ascontiguousarray(x_K[b].T),
            "xvT": np.ascontiguousarray(x_V[b].T),
            "wq": Wq, "wk": Wk, "wv": Wv, "wo": Wo,
            "bq8": bq8, "bo2": bo2,
            "maskb": np.ascontiguousarray(mvec.reshape(KC, P).T),
        })

    res = run_bass_kernel_spmd(nc, in_maps, core_ids=list(range(8)))

    out = np.empty((B, S, D), dtype=np.float32)
    for c in range(8):
        b, hh = c // 2, c % 2
        q0 = hh * SQ
        out[b, q0:q0 + SQ, :] = res.results[c]["yT"].T
    return out


# revision 27
# speedup vs baseline: 1.3330x; 1.0236x over previous
"""Trainium2 Bass kernel for nn_MultiHeadAttention (B=4, S=2048, D=1024, H=16, DH=64).

Sharding: 8 cores = 4 batches x 2 query-halves. Each core computes, for its
(batch b, query half): Q/K/V projections, masked softmax attention over the
full key length, and the output projection, entirely on-device.

Device-side layout is fully transposed (feature-major) so every matmul has its
contraction on the partition dim:
  QT = Wq^T xqT / 8         [HDH, Sq]   (1/8 score scale + bq folded in)
  KT = Wk^T xkT             [HDH, S]    (spilled to DRAM, per-pair reload)
  V  = (Wv^T xvT)^T         [S, HDH]    stored interleaved per head as
                                        [s, h, 65] with a ones column, so the
                                        PV matmul's PSUM row 64 accumulates
                                        l = sum_k exp(scoresT) for free
  scoresT_h = KT_h^T QT_h   [S, Sq]     (per head; 2 heads packed in PE rows)
  expT = exp(scoresT + mask_bias[k])    (no row-max: |scores| <= ~4)
  outT_h = V_h^T expT / l   [DH, Sq]
  yT = Wo^T outT + bo'      [D, Sq]
Key-padding mask enters as a per-partition bias (0 / -40) on the Exp
activation. bk is dropped (softmax-invariant); bv,bo fold into bo' = bv@Wo+bo
host-side (exact, since softmax rows sum to 1).
"""

import os
import sys
import numpy as np

if "/opt/trn_rl_repo" not in sys.path:
    sys.path.insert(0, "/opt/trn_rl_repo")

import concourse.bass as bass
import concourse.mybir as mybir
import concourse.tile as tile
from concourse import bacc
from concourse.bass_utils import run_bass_kernel_spmd

B, S, D = 4, 2048, 1024
H, DH = 16, 64
HDH = H * DH                      # 1024
SQ = S // 2                       # 1024 queries per core
P = 128
DC = D // P                       # 8 contraction chunks
NJ = 8                            # head pairs (2 heads x 64 rows = 128)
KC = S // P                       # 16 key chunks
SC = S // P                       # 16 s chunks for V
VW = DH + 1                       # 65: V columns per head + ones column
F32 = mybir.dt.float32
F32R = mybir.dt.float32r
BF16 = mybir.dt.bfloat16
MASK_NEG = -40.0

_CACHE = {}


def build_bass(kc_lim=KC):
    nc = bacc.Bacc("TRN2", target_bir_lowering=False, debug=False)
    klen = kc_lim * P                # keys actually attended (rest fully masked)
    kpad = ((klen + 511) // 512) * 512   # KT buffers padded to 512-col blocks

    xqT = nc.dram_tensor("xqT", [D, SQ], F32R, kind="ExternalInput").ap()
    xkT = nc.dram_tensor("xkT", [D, S], F32R, kind="ExternalInput").ap()
    xvT = nc.dram_tensor("xvT", [D, S], F32R, kind="ExternalInput").ap()
    wq = nc.dram_tensor("wq", [D, HDH], F32R, kind="ExternalInput").ap()
    wk = nc.dram_tensor("wk", [D, HDH], F32R, kind="ExternalInput").ap()
    wv = nc.dram_tensor("wv", [D, HDH], F32R, kind="ExternalInput").ap()
    wo = nc.dram_tensor("wo", [HDH, D], F32R, kind="ExternalInput").ap()
    bq8 = nc.dram_tensor("bq8", [P, NJ], F32, kind="ExternalInput").ap()
    bo2 = nc.dram_tensor("bo2", [P, DC], F32, kind="ExternalInput").ap()
    maskb = nc.dram_tensor("maskb", [P, KC], F32, kind="ExternalInput").ap()
    yT = nc.dram_tensor("yT", [D, SQ], F32, kind="ExternalOutput").ap()

    Exp = mybir.ActivationFunctionType.Exp
    AOp = mybir.AluOpType

    with tile.TileContext(nc) as tc:
        with (
            tc.tile_pool(name="const", bufs=1) as cpool,
            tc.tile_pool(name="vres", bufs=1) as vpool,
            tc.tile_pool(name="ktdram", bufs=1, space="DRAM") as ktd,
            tc.tile_pool(name="rdram", bufs=2, space="DRAM") as rdp,
        ):
            maskb_sb = cpool.tile([P, KC], F32)
            nc.sync.dma_start(out=maskb_sb, in_=maskb)
            bq8_sb = cpool.tile([P, NJ], F32)
            nc.sync.dma_start(out=bq8_sb, in_=bq8)
            bo2_sb = cpool.tile([P, DC], F32)
            nc.sync.dma_start(out=bo2_sb, in_=bo2)

            # V interleaved per head: [p, sc, h, 65]; col 64 of each head = 1.0
            v_sb = vpool.tile([P, SC, H, VW], F32R)
            kt_dram = [
                ktd.tile([P, kpad], F32R, tag=f"ktd{j}", name=f"ktd{j}")
                for j in range(NJ)
            ]

            # ---- phase V: V = (Wv^T xvT)^T, all pairs --------------------
            # x loaded in column blocks so compute starts after block 0 and
            # freed blocks let the next phase's loads overlap this compute.
            with (
                tc.tile_pool(name="xv", bufs=1) as xvp,
                tc.tile_pool(name="wvp", bufs=1) as wvp,
                tc.tile_pool(name="pv", bufs=2, space="PSUM") as pvp,
            ):
                wv_sb = wvp.tile([P, DC, HDH], F32R)
                wv_ch = wv.rearrange("(c p) n -> p c n", p=P)
                xv_cb = []
                xv_ch = xvT.rearrange("(c p) s -> p c s", p=P)
                for cb in range(4):
                    t = xvp.tile([P, DC, 512], F32R, tag=f"xv{cb}",
                                 name=f"xv{cb}")
                    xv_cb.append(t)
                nc.sync.dma_start(
                    out=xv_cb[0], in_=xv_ch[:, :, 0:512]
                )
                for kc in range(DC):
                    nc.scalar.dma_start(out=wv_sb[:, kc, :], in_=wv_ch[:, kc, :])
                for cb in range(1, 4):
                    nc.sync.dma_start(
                        out=xv_cb[cb], in_=xv_ch[:, :, cb * 512:(cb + 1) * 512]
                    )
                for sc in range(kc_lim):
                    cb, scl = sc // 4, sc % 4
                    ps = pvp.tile([P, HDH], F32, tag="pv")
                    for kc in range(DC):
                        lhsT = xv_cb[cb][:, kc, scl * P:(scl + 1) * P]
                        for nh in range(2):
                            nc.tensor.matmul(
                                ps[:, nh * 512:(nh + 1) * 512],
                                lhsT,
                                wv_sb[:, kc, nh * 512:(nh + 1) * 512],
                                start=(kc == 0),
                                stop=(kc == DC - 1),
                            )
                    # strided copy into the interleaved layout
                    nc.vector.tensor_copy(
                        v_sb[:, sc, :, 0:DH],
                        ps.rearrange("p (h d) -> p h d", d=DH),
                    )
                    nc.vector.tensor_scalar(
                        v_sb[:, sc, :, DH:VW], v_sb[:, sc, :, 0:1],
                        0.0, 1.0, AOp.mult, AOp.add,
                    )

            # ---- phase K: KT -> DRAM, all pairs --------------------------
            with (
                tc.tile_pool(name="wkp", bufs=1) as wkp,
                tc.tile_pool(name="xk", bufs=1) as xkp,
                tc.tile_pool(name="ktst", bufs=3) as ktstp,
                tc.tile_pool(name="pk", bufs=2, space="PSUM") as pkp,
            ):
                wk_sb = wkp.tile([P, DC, HDH], F32R)
                wk_ch = wk.rearrange("(c p) n -> p c n", p=P)
                for kc in range(DC):
                    eng = nc.scalar if kc % 2 == 0 else nc.sync
                    eng.dma_start(out=wk_sb[:, kc, :], in_=wk_ch[:, kc, :])
                xk_cb = []
                xk_ch = xkT.rearrange("(c p) s -> p c s", p=P)
                for cb in range(4):
                    t = xkp.tile([P, DC, 512], F32R, tag=f"xk{cb}",
                                 name=f"xk{cb}")
                    eng = nc.sync if cb % 2 == 0 else nc.scalar
                    eng.dma_start(
                        out=t, in_=xk_ch[:, :, cb * 512:(cb + 1) * 512]
                    )
                    xk_cb.append(t)
                n_kb = kpad // 512   # 512-col blocks of KT kept
                for half in range(2):
                    blks = [b for b in (2 * half, 2 * half + 1) if b < n_kb]
                    if not blks:
                        continue
                    w_cols = 512 * len(blks)
                    for j in range(NJ):
                        ps = pkp.tile([P, SQ], F32, tag="pk")
                        for kc in range(DC):
                            lhsT = wk_sb[:, kc, j * P:(j + 1) * P]
                            for bi, b in enumerate(blks):
                                nc.tensor.matmul(
                                    ps[:, bi * 512:(bi + 1) * 512],
                                    lhsT,
                                    xk_cb[b][:, kc, :],
                                    start=(kc == 0),
                                    stop=(kc == DC - 1),
                                )
                        st = ktstp.tile([P, SQ], F32R, tag="ktst")
                        nc.vector.tensor_copy(st[:, 0:w_cols], ps[:, 0:w_cols])
                        nc.gpsimd.dma_start(
                            out=kt_dram[j][:, half * SQ:half * SQ + w_cols],
                            in_=st[:, 0:w_cols],
                        )

            # ---- phase Q: QT resident, all pairs -------------------------
            with tc.tile_pool(name="qtres", bufs=1) as qtpool:
                qt = [
                    qtpool.tile([P, SQ], F32R, tag=f"qt{j}", name=f"qt{j}")
                    for j in range(NJ)
                ]
                with (
                    tc.tile_pool(name="wqp", bufs=1) as wqp,
                    tc.tile_pool(name="xq", bufs=1) as xqp,
                    tc.tile_pool(name="pq", bufs=2, space="PSUM") as pqp,
                ):
                    wq_sb = wqp.tile([P, DC, HDH], F32R)
                    wq_ch = wq.rearrange("(c p) n -> p c n", p=P)
                    for kc in range(DC):
                        eng = nc.scalar if kc % 2 == 0 else nc.sync
                        eng.dma_start(out=wq_sb[:, kc, :], in_=wq_ch[:, kc, :])
                    xq_cb = []
                    xq_ch = xqT.rearrange("(c p) s -> p c s", p=P)
                    for cb in range(2):
                        t = xqp.tile([P, DC, 512], F32R, tag=f"xq{cb}",
                                     name=f"xq{cb}")
                        eng = nc.sync if cb % 2 == 0 else nc.scalar
                        eng.dma_start(
                            out=t, in_=xq_ch[:, :, cb * 512:(cb + 1) * 512]
                        )
                        xq_cb.append(t)
                    for j in range(NJ):
                        ps = pqp.tile([P, SQ], F32, tag="pq")
                        for kc in range(DC):
                            lhsT = wq_sb[:, kc, j * P:(j + 1) * P]
                            for nh in range(2):
                                nc.tensor.matmul(
                                    ps[:, nh * 512:(nh + 1) * 512],
                                    lhsT,
                                    xq_cb[nh][:, kc, :],
                                    start=(kc == 0),
                                    stop=(kc == DC - 1),
                                )
                        nc.vector.tensor_scalar(
                            qt[j], ps, 0.125, bq8_sb[:, j:j + 1],
                            AOp.mult, AOp.add,
                        )

                # ---- attention -------------------------------------------
                with tc.tile_pool(name="otres", bufs=1) as otpool:
                    ot = [
                        otpool.tile([P, SQ], F32R, tag=f"ot{j}", name=f"ot{j}")
                        for j in range(NJ)
                    ]
                    with (
                        tc.tile_pool(name="ktsb", bufs=2) as ktp,
                        tc.tile_pool(name="expp", bufs=3) as expp,
                        tc.tile_pool(name="lbp", bufs=2) as lbp,
                        tc.tile_pool(name="ps_s", bufs=1, space="PSUM") as pss,
                        tc.tile_pool(name="ps_o", bufs=1, space="PSUM") as pso,
                    ):
                        kt_tiles = {}

                        def load_kt(jj):
                            t = ktp.tile([P, kpad], F32R, tag="kt", name="kt_sb")
                            nc.sync.dma_start(out=t, in_=kt_dram[jj][:])
                            kt_tiles[jj] = t

                        load_kt(0)
                        for j in range(NJ):
                            if j + 1 < NJ:
                                load_kt(j + 1)  # prefetch next pair's KT
                            kt_sb = kt_tiles.pop(j)
                            ps_oa = pso.tile([VW, SQ], F32, tag="oa")
                            ps_ob = pso.tile([VW, SQ], F32, tag="ob")
                            ets = {}

                            def scores_exp(kc, j=j, kt_sb=kt_sb, ets=ets):
                                ps_s = pss.tile(
                                    [P, 2 * SQ], F32, tag="s", name="ps_s"
                                )
                                for hh in (0, 1):
                                    lhsT = kt_sb[hh * 64:(hh + 1) * 64,
                                                 kc * P:(kc + 1) * P]
                                    for nh in range(2):
                                        nc.tensor.matmul(
                                            ps_s[:, hh * SQ + nh * 512:
                                                 hh * SQ + (nh + 1) * 512],
                                            lhsT,
                                            qt[j][hh * 64:(hh + 1) * 64,
                                                  nh * 512:(nh + 1) * 512],
                                            tile_position=(hh * 64, 0),
                                        )
                                et = expp.tile(
                                    [P, 2 * SQ], F32R, tag="e", name="et"
                                )
                                nc.scalar.activation(
                                    et, ps_s, Exp,
                                    bias=maskb_sb[:, kc:kc + 1], scale=1.0,
                                )
                                ets[kc] = et

                            def pv(kc, j=j, ets=ets, ps_oa=ps_oa, ps_ob=ps_ob):
                                et = ets.pop(kc)
                                for hh, ps_o in ((0, ps_oa), (1, ps_ob)):
                                    vh = v_sb[:, kc, 2 * j + hh, :]  # [128,65]
                                    for nh in range(2):
                                        nc.tensor.matmul(
                                            ps_o[:, nh * 512:(nh + 1) * 512],
                                            vh,
                                            et[:, hh * SQ + nh * 512:
                                               hh * SQ + (nh + 1) * 512],
                                            start=(kc == 0),
                                            stop=(kc == kc_lim - 1),
                                        )

                            scores_exp(0)
                            scores_exp(1)
                            for kc in range(2, kc_lim):
                                scores_exp(kc)
                                pv(kc - 2)
                            pv(kc_lim - 2)
                            pv(kc_lim - 1)

                            # release PSUM fast: copy both heads to SBUF
                            # (incl. l rows); broadcast l, recip, then scale
                            cpA = lbp.tile([VW, SQ], F32, tag="cpA", bufs=2)
                            nc.vector.tensor_copy(cpA, ps_oa)
                            cpB = lbp.tile([VW, SQ], F32R, tag="cpB", bufs=2)
                            nc.vector.tensor_copy(cpB, ps_ob)
                            L_sb = lbp.tile([P, SQ], F32, tag="L", bufs=2)
                            for hh, rsrc in ((0, cpA), (1, cpB)):
                                rd = rdp.tile(
                                    [1, SQ], F32, tag="rd", name="rd"
                                )
                                nc.sync.dma_start(
                                    out=rd, in_=rsrc[DH:VW, :].bitcast(F32)
                                )
                                rd_b = bass.AP(
                                    tensor=rd.tensor, offset=rd.offset,
                                    ap=[[0, 64], rd.ap[-1]],
                                )
                                nc.sync.dma_start(
                                    out=L_sb[hh * 64:(hh + 1) * 64, :],
                                    in_=rd_b,
                                )
                            nc.vector.reciprocal_approx_fast(L_sb, L_sb)
                            nc.vector.tensor_mul(
                                ot[j][0:64, :], cpA[0:DH, :], L_sb[0:64, :]
                            )
                            nc.gpsimd.dma_start(
                                out=ot[j][64:128, :], in_=cpB[0:DH, :]
                            )
                            nc.vector.tensor_mul(
                                ot[j][64:128, :], ot[j][64:128, :],
                                L_sb[64:128, :],
                            )

                    # ---- output projection -------------------------------
                    with (
                        tc.tile_pool(name="wop", bufs=8) as wop,
                        tc.tile_pool(name="ytp", bufs=3) as ytp,
                        tc.tile_pool(name="py", bufs=2, space="PSUM") as pyp,
                    ):
                        yt_ch = yT.rearrange("(c p) s -> c p s", p=P)
                        for dc in range(DC):
                            ps = pyp.tile([P, SQ], F32, tag="py")
                            for j in range(NJ):
                                wo_t = wop.tile([P, P], F32R, tag="wo")
                                nc.scalar.dma_start(
                                    out=wo_t,
                                    in_=wo[j * P:(j + 1) * P,
                                           dc * P:(dc + 1) * P],
                                )
                                for nh in range(2):
                                    nc.tensor.matmul(
                                        ps[:, nh * 512:(nh + 1) * 512],
                                        wo_t,
                                        ot[j][:, nh * 512:(nh + 1) * 512],
                                        start=(j == 0),
                                        stop=(j == NJ - 1),
                                    )
                            yt_sb = ytp.tile([P, SQ], F32, tag="yt")
                            nc.vector.tensor_scalar(
                                yt_sb, ps, bo2_sb[:, dc:dc + 1], None, AOp.add
                            )
                            nc.gpsimd.dma_start(out=yt_ch[dc], in_=yt_sb)

    nc.compile()
    return nc


def kernel(x_Q, x_K, x_V, src_batch_lens, Wq, bq, Wk, bk, Wv, bv, Wo, bo):
    x_Q = np.asarray(x_Q, dtype=np.float32)
    x_K = np.asarray(x_K, dtype=np.float32)
    x_V = np.asarray(x_V, dtype=np.float32)
    lens = np.asarray(src_batch_lens)
    Wq = np.ascontiguousarray(np.asarray(Wq, dtype=np.float32))
    Wk = np.ascontiguousarray(np.asarray(Wk, dtype=np.float32))
    Wv = np.ascontiguousarray(np.asarray(Wv, dtype=np.float32))
    Wo = np.ascontiguousarray(np.asarray(Wo, dtype=np.float32))
    bq = np.asarray(bq, dtype=np.float32)
    bv = np.asarray(bv, dtype=np.float32)
    bo = np.asarray(bo, dtype=np.float32)

    maxlen = int(np.max(lens))
    maxlen = max(1, min(S, maxlen))
    kc_lim = (maxlen + P - 1) // P
    if kc_lim not in _CACHE:
        _CACHE[kc_lim] = build_bass(kc_lim)
    nc = _CACHE[kc_lim]

    bo2_full = (bv @ Wo + bo).astype(np.float32)
    bo2 = np.ascontiguousarray(bo2_full.reshape(DC, P).T)
    bq8 = np.ascontiguousarray((bq / 8.0).reshape(NJ, P).T)

    in_maps = []
    for c in range(8):
        b, hh = c // 2, c % 2
        q0 = hh * SQ
        k_idx = np.arange(S)
        mvec = np.where(k_idx < int(lens[b]), 0.0, MASK_NEG).astype(np.float32)
        in_maps.append({
            "xqT": np.ascontiguousarray(x_Q[b, q0:q0 + SQ, :].T),
            "xkT": np.ascontiguousarray(x_K[b].T),
            "xvT": np.ascontiguousarray(x_V[b].T),
            "wq": Wq, "wk": Wk, "wv": Wv, "wo": Wo,
            "bq8": bq8, "bo2": bo2,
            "maskb": np.ascontiguousarray(mvec.reshape(KC, P).T),
        })

    res = run_bass_kernel_spmd(nc, in_maps, core_ids=list(range(8)))

    out = np.empty((B, S, D), dtype=np.float32)
    for c in range(8):
        b, hh = c // 2, c % 2
        q0 = hh * SQ
        out[b, q0:q0 + SQ, :] = res.results[c]["yT"].T
    return out


# revision 28
# speedup vs baseline: 1.3400x; 1.0053x over previous
"""Trainium2 Bass kernel for nn_MultiHeadAttention (B=4, S=2048, D=1024, H=16, DH=64).

Sharding: 8 cores = 4 batches x 2 query-halves. Each core computes, for its
(batch b, query half): Q/K/V projections, masked softmax attention over the
full key length, and the output projection, entirely on-device.

Device-side layout is fully transposed (feature-major) so every matmul has its
contraction on the partition dim:
  QT = Wq^T xqT / 8         [HDH, Sq]   (1/8 score scale + bq folded in)
  KT = Wk^T xkT             [HDH, S]    (spilled to DRAM, per-pair reload)
  V  = (Wv^T xvT)^T         [S, HDH]    stored interleaved per head as
                                        [s, h, 65] with a ones column, so the
                                        PV matmul's PSUM row 64 accumulates
                                        l = sum_k exp(scoresT) for free
  scoresT_h = KT_h^T QT_h   [S, Sq]     (per head; 2 heads packed in PE rows)
  expT = exp(scoresT + mask_bias[k])    (no row-max: |scores| <= ~4)
  outT_h = V_h^T expT / l   [DH, Sq]
  yT = Wo^T outT + bo'      [D, Sq]
Key-padding mask enters as a per-partition bias (0 / -40) on the Exp
activation. bk is dropped (softmax-invariant); bv,bo fold into bo' = bv@Wo+bo
host-side (exact, since softmax rows sum to 1).
"""

import os
import sys
import numpy as np

if "/opt/trn_rl_repo" not in sys.path:
    sys.path.insert(0, "/opt/trn_rl_repo")

import concourse.bass as bass
import concourse.mybir as mybir
import concourse.tile as tile
from concourse import bacc
from concourse.bass_utils import run_bass_kernel_spmd

B, S, D = 4, 2048, 1024
H, DH = 16, 64
HDH = H * DH                      # 1024
SQ = S // 2                       # 1024 queries per core
P = 128
DC = D // P                       # 8 contraction chunks
NJ = 8                            # head pairs (2 heads x 64 rows = 128)
KC = S // P                       # 16 key chunks
SC = S // P                       # 16 s chunks for V
VW = DH + 1                       # 65: V columns per head + ones column
F32 = mybir.dt.float32
F32R = mybir.dt.float32r
BF16 = mybir.dt.bfloat16
MASK_NEG = -40.0

_CACHE = {}


def build_bass(kc_lim=KC):
    nc = bacc.Bacc("TRN2", target_bir_lowering=False, debug=False)
    klen = kc_lim * P                # keys actually attended (rest fully masked)
    kpad = ((klen + 511) // 512) * 512   # KT buffers padded to 512-col blocks

    xqT = nc.dram_tensor("xqT", [D, SQ], F32R, kind="ExternalInput").ap()
    xkT = nc.dram_tensor("xkT", [D, S], F32R, kind="ExternalInput").ap()
    xvT = nc.dram_tensor("xvT", [D, S], F32R, kind="ExternalInput").ap()
    wq = nc.dram_tensor("wq", [D, HDH], F32R, kind="ExternalInput").ap()
    wk = nc.dram_tensor("wk", [D, HDH], F32R, kind="ExternalInput").ap()
    wv = nc.dram_tensor("wv", [D, HDH], F32R, kind="ExternalInput").ap()
    wo = nc.dram_tensor("wo", [HDH, D], F32R, kind="ExternalInput").ap()
    bq8 = nc.dram_tensor("bq8", [P, NJ], F32, kind="ExternalInput").ap()
    bo2 = nc.dram_tensor("bo2", [P, DC], F32, kind="ExternalInput").ap()
    maskb = nc.dram_tensor("maskb", [P, KC], F32, kind="ExternalInput").ap()
    yT = nc.dram_tensor("yT", [D, SQ], F32, kind="ExternalOutput").ap()

    Exp = mybir.ActivationFunctionType.Exp
    AOp = mybir.AluOpType

    with tile.TileContext(nc) as tc:
        with (
            tc.tile_pool(name="const", bufs=1) as cpool,
            tc.tile_pool(name="vres", bufs=1) as vpool,
            tc.tile_pool(name="ktdram", bufs=1, space="DRAM") as ktd,
            tc.tile_pool(name="rdram", bufs=2, space="DRAM") as rdp,
        ):
            maskb_sb = cpool.tile([P, KC], F32)
            nc.sync.dma_start(out=maskb_sb, in_=maskb)
            bq8_sb = cpool.tile([P, NJ], F32)
            nc.sync.dma_start(out=bq8_sb, in_=bq8)
            bo2_sb = cpool.tile([P, DC], F32)
            nc.sync.dma_start(out=bo2_sb, in_=bo2)

            # V interleaved per head: [p, sc, h, 65]; col 64 of each head = 1.0
            v_sb = vpool.tile([P, SC, H, VW], F32R)
            kt_dram = [
                ktd.tile([P, kpad], F32R, tag=f"ktd{j}", name=f"ktd{j}")
                for j in range(NJ)
            ]

            # ---- phase V: V = (Wv^T xvT)^T, all pairs --------------------
            # x loaded in column blocks so compute starts after block 0 and
            # freed blocks let the next phase's loads overlap this compute.
            with (
                tc.tile_pool(name="xv", bufs=1) as xvp,
                tc.tile_pool(name="wvp", bufs=1) as wvp,
                tc.tile_pool(name="pv", bufs=2, space="PSUM") as pvp,
            ):
                wv_sb = wvp.tile([P, DC, HDH], F32R)
                wv_ch = wv.rearrange("(c p) n -> p c n", p=P)
                xv_cb = []
                xv_ch = xvT.rearrange("(c p) s -> p c s", p=P)
                for cb in range(4):
                    t = xvp.tile([P, DC, 512], F32R, tag=f"xv{cb}",
                                 name=f"xv{cb}")
                    xv_cb.append(t)
                nc.sync.dma_start(
                    out=xv_cb[0], in_=xv_ch[:, :, 0:512]
                )
                for kc in range(DC):
                    nc.scalar.dma_start(out=wv_sb[:, kc, :], in_=wv_ch[:, kc, :])
                for cb in range(1, 4):
                    nc.sync.dma_start(
                        out=xv_cb[cb], in_=xv_ch[:, :, cb * 512:(cb + 1) * 512]
                    )
                for sc in range(kc_lim):
                    cb, scl = sc // 4, sc % 4
                    ps = pvp.tile([P, HDH], F32, tag="pv")
                    for kc in range(DC):
                        lhsT = xv_cb[cb][:, kc, scl * P:(scl + 1) * P]
                        for nh in range(2):
                            nc.tensor.matmul(
                                ps[:, nh * 512:(nh + 1) * 512],
                                lhsT,
                                wv_sb[:, kc, nh * 512:(nh + 1) * 512],
                                start=(kc == 0),
                                stop=(kc == DC - 1),
                            )
                    # strided copy into the interleaved layout
                    nc.vector.tensor_copy(
                        v_sb[:, sc, :, 0:DH],
                        ps.rearrange("p (h d) -> p h d", d=DH),
                    )
                    nc.vector.tensor_scalar(
                        v_sb[:, sc, :, DH:VW], v_sb[:, sc, :, 0:1],
                        0.0, 1.0, AOp.mult, AOp.add,
                    )

            # ---- phase K: KT -> DRAM, all pairs --------------------------
            with (
                tc.tile_pool(name="xk", bufs=1) as xkp,
                tc.tile_pool(name="wkp", bufs=1) as wkp,
                tc.tile_pool(name="ktst", bufs=3) as ktstp,
                tc.tile_pool(name="pk", bufs=2, space="PSUM") as pkp,
            ):
                xk_cb = []
                xk_ch = xkT.rearrange("(c p) s -> p c s", p=P)
                for cb in range(4):
                    t = xkp.tile([P, DC, 512], F32R, tag=f"xk{cb}",
                                 name=f"xk{cb}")
                    eng = nc.sync if cb % 2 == 0 else nc.scalar
                    eng.dma_start(
                        out=t, in_=xk_ch[:, :, cb * 512:(cb + 1) * 512]
                    )
                    xk_cb.append(t)
                wk_sb = wkp.tile([P, DC, HDH], F32R)
                wk_ch = wk.rearrange("(c p) n -> p c n", p=P)
                for kc in range(DC):
                    eng = nc.scalar if kc % 2 == 0 else nc.sync
                    eng.dma_start(out=wk_sb[:, kc, :], in_=wk_ch[:, kc, :])
                n_kb = kpad // 512   # 512-col blocks of KT kept
                for half in range(2):
                    blks = [b for b in (2 * half, 2 * half + 1) if b < n_kb]
                    if not blks:
                        continue
                    w_cols = 512 * len(blks)
                    for j in range(NJ):
                        ps = pkp.tile([P, SQ], F32, tag="pk")
                        for kc in range(DC):
                            lhsT = wk_sb[:, kc, j * P:(j + 1) * P]
                            for bi, b in enumerate(blks):
                                nc.tensor.matmul(
                                    ps[:, bi * 512:(bi + 1) * 512],
                                    lhsT,
                                    xk_cb[b][:, kc, :],
                                    start=(kc == 0),
                                    stop=(kc == DC - 1),
                                )
                        st = ktstp.tile([P, SQ], F32R, tag="ktst")
                        nc.vector.tensor_copy(st[:, 0:w_cols], ps[:, 0:w_cols])
                        nc.gpsimd.dma_start(
                            out=kt_dram[j][:, half * SQ:half * SQ + w_cols],
                            in_=st[:, 0:w_cols],
                        )

            # ---- phase Q: QT resident, all pairs -------------------------
            with tc.tile_pool(name="qtres", bufs=1) as qtpool:
                qt = [
                    qtpool.tile([P, SQ], F32R, tag=f"qt{j}", name=f"qt{j}")
                    for j in range(NJ)
                ]
                with (
                    tc.tile_pool(name="xq", bufs=1) as xqp,
                    tc.tile_pool(name="wqp", bufs=1) as wqp,
                    tc.tile_pool(name="pq", bufs=2, space="PSUM") as pqp,
                ):
                    xq_cb = []
                    xq_ch = xqT.rearrange("(c p) s -> p c s", p=P)
                    for cb in range(2):
                        t = xqp.tile([P, DC, 512], F32R, tag=f"xq{cb}",
                                     name=f"xq{cb}")
                        eng = nc.sync if cb % 2 == 0 else nc.scalar
                        eng.dma_start(
                            out=t, in_=xq_ch[:, :, cb * 512:(cb + 1) * 512]
                        )
                        xq_cb.append(t)
                    wq_sb = wqp.tile([P, DC, HDH], F32R)
                    wq_ch = wq.rearrange("(c p) n -> p c n", p=P)
                    for kc in range(DC):
                        eng = nc.scalar if kc % 2 == 0 else nc.sync
                        eng.dma_start(out=wq_sb[:, kc, :], in_=wq_ch[:, kc, :])
                    for j in range(NJ):
                        ps = pqp.tile([P, SQ], F32, tag="pq")
                        for kc in range(DC):
                            lhsT = wq_sb[:, kc, j * P:(j + 1) * P]
                            for nh in range(2):
                                nc.tensor.matmul(
                                    ps[:, nh * 512:(nh + 1) * 512],
                                    lhsT,
                                    xq_cb[nh][:, kc, :],
                                    start=(kc == 0),
                                    stop=(kc == DC - 1),
                                )
                        nc.vector.tensor_scalar(
                            qt[j], ps, 0.125, bq8_sb[:, j:j + 1],
                            AOp.mult, AOp.add,
                        )

                # ---- attention -------------------------------------------
                with (
                    tc.tile_pool(name="otres", bufs=1) as otpool,
                    tc.tile_pool(name="wopre", bufs=1) as wopre,
                ):
                    wo_pre = wopre.tile([P, 16, P], F32R)
                    for jj in range(NJ):
                        nc.sync.dma_start(
                            out=wo_pre[:, jj, :],
                            in_=wo[jj * P:(jj + 1) * P, 0:P],
                        )
                        nc.sync.dma_start(
                            out=wo_pre[:, NJ + jj, :],
                            in_=wo[jj * P:(jj + 1) * P, P:2 * P],
                        )
                    ot = [
                        otpool.tile([P, SQ], F32R, tag=f"ot{j}", name=f"ot{j}")
                        for j in range(NJ)
                    ]
                    with (
                        tc.tile_pool(name="ktsb", bufs=2) as ktp,
                        tc.tile_pool(name="expp", bufs=3) as expp,
                        tc.tile_pool(name="lbp", bufs=2) as lbp,
                        tc.tile_pool(name="ps_s", bufs=1, space="PSUM") as pss,
                        tc.tile_pool(name="ps_o", bufs=1, space="PSUM") as pso,
                    ):
                        kt_tiles = {}

                        def load_kt(jj):
                            t = ktp.tile([P, kpad], F32R, tag="kt", name="kt_sb")
                            nc.sync.dma_start(out=t, in_=kt_dram[jj][:])
                            kt_tiles[jj] = t

                        load_kt(0)
                        for j in range(NJ):
                            if j + 1 < NJ:
                                load_kt(j + 1)  # prefetch next pair's KT
                            kt_sb = kt_tiles.pop(j)
                            ps_oa = pso.tile([VW, SQ], F32, tag="oa")
                            ps_ob = pso.tile([VW, SQ], F32, tag="ob")
                            ets = {}

                            def scores_exp(kc, j=j, kt_sb=kt_sb, ets=ets):
                                ps_s = pss.tile(
                                    [P, 2 * SQ], F32, tag="s", name="ps_s"
                                )
                                for hh in (0, 1):
                                    lhsT = kt_sb[hh * 64:(hh + 1) * 64,
                                                 kc * P:(kc + 1) * P]
                                    for nh in range(2):
                                        nc.tensor.matmul(
                                            ps_s[:, hh * SQ + nh * 512:
                                                 hh * SQ + (nh + 1) * 512],
                                            lhsT,
                                            qt[j][hh * 64:(hh + 1) * 64,
                                                  nh * 512:(nh + 1) * 512],
                                            tile_position=(hh * 64, 0),
                                        )
                                et = expp.tile(
                                    [P, 2 * SQ], F32R, tag="e", name="et"
                                )
                                nc.scalar.activation(
                                    et, ps_s, Exp,
                                    bias=maskb_sb[:, kc:kc + 1], scale=1.0,
                                )
                                ets[kc] = et

                            def pv(kc, j=j, ets=ets, ps_oa=ps_oa, ps_ob=ps_ob):
                                et = ets.pop(kc)
                                for hh, ps_o in ((0, ps_oa), (1, ps_ob)):
                                    vh = v_sb[:, kc, 2 * j + hh, :]  # [128,65]
                                    for nh in range(2):
                                        nc.tensor.matmul(
                                            ps_o[:, nh * 512:(nh + 1) * 512],
                                            vh,
                                            et[:, hh * SQ + nh * 512:
                                               hh * SQ + (nh + 1) * 512],
                                            start=(kc == 0),
                                            stop=(kc == kc_lim - 1),
                                        )

                            scores_exp(0)
                            scores_exp(1)
                            for kc in range(2, kc_lim):
                                scores_exp(kc)
                                pv(kc - 2)
                            pv(kc_lim - 2)
                            pv(kc_lim - 1)

                            # release PSUM fast: copy both heads to SBUF
                            # (incl. l rows); broadcast l, recip, then scale
                            cpA = lbp.tile([VW, SQ], F32, tag="cpA", bufs=2)
                            nc.vector.tensor_copy(cpA, ps_oa)
                            cpB = lbp.tile([VW, SQ], F32R, tag="cpB", bufs=2)
                            nc.vector.tensor_copy(cpB, ps_ob)
                            L_sb = lbp.tile([P, SQ], F32, tag="L", bufs=2)
                            for hh, rsrc in ((0, cpA), (1, cpB)):
                                rd = rdp.tile(
                                    [1, SQ], F32, tag="rd", name="rd"
                                )
                                nc.sync.dma_start(
                                    out=rd, in_=rsrc[DH:VW, :].bitcast(F32)
                                )
                                rd_b = bass.AP(
                                    tensor=rd.tensor, offset=rd.offset,
                                    ap=[[0, 64], rd.ap[-1]],
                                )
                                nc.sync.dma_start(
                                    out=L_sb[hh * 64:(hh + 1) * 64, :],
                                    in_=rd_b,
                                )
                            nc.vector.reciprocal_approx_fast(L_sb, L_sb)
                            nc.vector.tensor_mul(
                                ot[j][0:64, :], cpA[0:DH, :], L_sb[0:64, :]
                            )
                            nc.gpsimd.dma_start(
                                out=ot[j][64:128, :], in_=cpB[0:DH, :]
                            )
                            nc.vector.tensor_mul(
                                ot[j][64:128, :], ot[j][64:128, :],
                                L_sb[64:128, :],
                            )

                    # ---- output projection -------------------------------
                    with (
                        tc.tile_pool(name="wop", bufs=8) as wop,
                        tc.tile_pool(name="ytp", bufs=3) as ytp,
                        tc.tile_pool(name="py", bufs=2, space="PSUM") as pyp,
                    ):
                        yt_ch = yT.rearrange("(c p) s -> c p s", p=P)
                        for dc in range(DC):
                            ps = pyp.tile([P, SQ], F32, tag="py")
                            for j in range(NJ):
                                if dc < 2:
                                    wo_t = wo_pre[:, dc * NJ + j, :]
                                else:
                                    wo_t = wop.tile([P, P], F32R, tag="wo")
                                    nc.scalar.dma_start(
                                        out=wo_t,
                                        in_=wo[j * P:(j + 1) * P,
                                               dc * P:(dc + 1) * P],
                                    )
                                for nh in range(2):
                                    nc.tensor.matmul(
                                        ps[:, nh * 512:(nh + 1) * 512],
                                        wo_t,
                                        ot[j][:, nh * 512:(nh + 1) * 512],
                                        start=(j == 0),
                                        stop=(j == NJ - 1),
                                    )
                            yt_sb = ytp.tile([P, SQ], F32, tag="yt")
                            nc.vector.tensor_scalar(
                                yt_sb, ps, bo2_sb[:, dc:dc + 1], None, AOp.add
                            )
                            nc.gpsimd.dma_start(out=yt_ch[dc], in_=yt_sb)

    nc.compile()
    return nc


def kernel(x_Q, x_K, x_V, src_batch_lens, Wq, bq, Wk, bk, Wv, bv, Wo, bo):
    x_Q = np.asarray(x_Q, dtype=np.float32)
    x_K = np.asarray(x_K, dtype=np.float32)
    x_V = np.asarray(x_V, dtype=np.float32)
    lens = np.asarray(src_batch_lens)
    Wq = np.ascontiguousarray(np.asarray(Wq, dtype=np.float32))
    Wk = np.ascontiguousarray(np.asarray(Wk, dtype=np.float32))
    Wv = np.ascontiguousarray(np.asarray(Wv, dtype=np.float32))
    Wo = np.ascontiguousarray(np.asarray(Wo, dtype=np.float32))
    bq = np.asarray(bq, dtype=np.float32)
    bv = np.asarray(bv, dtype=np.float32)
    bo = np.asarray(bo, dtype=np.float32)

    maxlen = int(np.max(lens))
    maxlen = max(1, min(S, maxlen))
    kc_lim = (maxlen + P - 1) // P
    if kc_lim not in _CACHE:
        _CACHE[kc_lim] = build_bass(kc_lim)
    nc = _CACHE[kc_lim]

    bo2_full = (bv @ Wo + bo).astype(np.float32)
    bo2 = np.ascontiguousarray(bo2_full.reshape(DC, P).T)
    bq8 = np.ascontiguousarray((bq / 8.0).reshape(NJ, P).T)

    in_maps = []
    for c in range(8):
        b, hh = c // 2, c % 2
        q0 = hh * SQ
        k_idx = np.arange(S)
        mvec = np.where(k_idx < int(lens[b]), 0.0, MASK_NEG).astype(np.float32)
        in_maps.append({
            "xqT": np.ascontiguousarray(x_Q[b, q0:q0 + SQ, :].T),
            "xkT": np.ascontiguousarray(x_K[b].T),
            "xvT": np.ascontiguousarray(x_V[b].T),
            "wq": Wq, "wk": Wk, "wv": Wv, "wo": Wo,
            "bq8": bq8, "bo2": bo2,
            "maskb": np.ascontiguousarray(mvec.reshape(KC, P).T),
        })

    res = run_bass_kernel_spmd(nc, in_maps, core_ids=list(range(8)))

    out = np.empty((B, S, D), dtype=np.float32)
    for c in range(8):
        b, hh = c // 2, c % 2
        q0 = hh * SQ
        out[b, q0:q0 + SQ, :] = res.results[c]["yT"].T
    return out


# revision 29
# speedup vs baseline: 1.4497x; 1.0819x over previous
"""Trainium2 Bass kernel for nn_MultiHeadAttention (B=4, S=2048, D=1024, H=16, DH=64).

Sharding: 8 cores = 4 batches x 2 query-halves. Each core computes, for its
(batch b, query half): Q/K/V projections, masked softmax attention over the
full key length, and the output projection, entirely on-device.

Device-side layout is fully transposed (feature-major) so every matmul has its
contraction on the partition dim:
  QT = Wq^T xqT / 8         [HDH, Sq]   (1/8 score scale + bq folded in)
  KT = Wk^T xkT             [HDH, S]    (spilled to DRAM, per-pair reload)
  V  = (Wv^T xvT)^T         [S, HDH]    stored interleaved per head as
                                        [s, h, 65] with a ones column, so the
                                        PV matmul's PSUM row 64 accumulates
                                        l = sum_k exp(scoresT) for free
  scoresT_h = KT_h^T QT_h   [S, Sq]     (per head; 2 heads packed in PE rows)
  expT = exp(scoresT + mask_bias[k])    (no row-max: |scores| <= ~4)
  outT_h = V_h^T expT / l   [DH, Sq]
  yT = Wo^T outT + bo'      [D, Sq]
Key-padding mask enters as a per-partition bias (0 / -40) on the Exp
activation. bk is dropped (softmax-invariant); bv,bo fold into bo' = bv@Wo+bo
host-side (exact, since softmax rows sum to 1).
"""

import os
import sys
import numpy as np

if "/opt/trn_rl_repo" not in sys.path:
    sys.path.insert(0, "/opt/trn_rl_repo")

import concourse.bass as bass
import concourse.mybir as mybir
import concourse.tile as tile
from concourse import bacc
from concourse.bass_utils import run_bass_kernel_spmd

B, S, D = 4, 2048, 1024
H, DH = 16, 64
HDH = H * DH                      # 1024
SQ = S // 2                       # 1024 queries per core
P = 128
DC = D // P                       # 8 contraction chunks
NJ = 8                            # head pairs (2 heads x 64 rows = 128)
KC = S // P                       # 16 key chunks
SC = S // P                       # 16 s chunks for V
VW = DH + 1                       # 65: V columns per head + ones column
F32 = mybir.dt.float32
F32R = mybir.dt.float32r
BF16 = mybir.dt.bfloat16
MASK_NEG = -40.0

_CACHE = {}


def build_bass(kc_lim=KC):
    nc = bacc.Bacc("TRN2", target_bir_lowering=False, debug=False)
    klen = kc_lim * P                # keys actually attended (rest fully masked)
    kpad = ((klen + 511) // 512) * 512   # KT buffers padded to 512-col blocks

    xqT = nc.dram_tensor("xqT", [D, SQ], F32R, kind="ExternalInput").ap()
    xkT = nc.dram_tensor("xkT", [D, S], F32R, kind="ExternalInput").ap()
    xvT = nc.dram_tensor("xvT", [D, S], F32R, kind="ExternalInput").ap()
    wq = nc.dram_tensor("wq", [D, HDH], F32R, kind="ExternalInput").ap()
    wk = nc.dram_tensor("wk", [D, HDH], F32R, kind="ExternalInput").ap()
    wv = nc.dram_tensor("wv", [D, HDH], F32R, kind="ExternalInput").ap()
    wo = nc.dram_tensor("wo", [HDH, D], F32R, kind="ExternalInput").ap()
    bq8 = nc.dram_tensor("bq8", [P, NJ], F32, kind="ExternalInput").ap()
    bo2 = nc.dram_tensor("bo2", [P, DC], F32, kind="ExternalInput").ap()
    maskb = nc.dram_tensor("maskb", [P, KC], F32, kind="ExternalInput").ap()
    yT = nc.dram_tensor("yT", [D, SQ], F32, kind="ExternalOutput").ap()

    Exp = mybir.ActivationFunctionType.Exp
    AOp = mybir.AluOpType

    with tile.TileContext(nc) as tc:
        with (
            tc.tile_pool(name="const", bufs=1) as cpool,
            tc.tile_pool(name="vres", bufs=1) as vpool,
            tc.tile_pool(name="ktdram", bufs=1, space="DRAM") as ktd,
            tc.tile_pool(name="rdram", bufs=2, space="DRAM") as rdp,
        ):
            maskb_sb = cpool.tile([P, KC], F32)
            nc.sync.dma_start(out=maskb_sb, in_=maskb)
            bq8_sb = cpool.tile([P, NJ], F32)
            nc.sync.dma_start(out=bq8_sb, in_=bq8)
            bo2_sb = cpool.tile([P, DC], F32)
            nc.sync.dma_start(out=bo2_sb, in_=bo2)

            # V interleaved per head: [p, sc, h, 65]; col 64 of each head = 1.0
            v_sb = vpool.tile([P, SC, H, VW], F32R)
            kt_dram = [
                ktd.tile([P, kpad], F32R, tag=f"ktd{j}", name=f"ktd{j}")
                for j in range(NJ)
            ]

            # ---- phase V: V = (Wv^T xvT)^T, all pairs --------------------
            # x loaded in column blocks so compute starts after block 0 and
            # freed blocks let the next phase's loads overlap this compute.
            with (
                tc.tile_pool(name="xv", bufs=1) as xvp,
                tc.tile_pool(name="wvp", bufs=1) as wvp,
                tc.tile_pool(name="pv", bufs=2, space="PSUM") as pvp,
            ):
                wv_sb = wvp.tile([P, DC, HDH], F32R)
                wv_ch = wv.rearrange("(c p) n -> p c n", p=P)
                xv_cb = []
                xv_ch = xvT.rearrange("(c p) s -> p c s", p=P)
                for cb in range(4):
                    t = xvp.tile([P, DC, 512], F32R, tag=f"xv{cb}",
                                 name=f"xv{cb}")
                    xv_cb.append(t)
                nc.sync.dma_start(
                    out=xv_cb[0], in_=xv_ch[:, :, 0:512]
                )
                for kc in range(DC):
                    nc.scalar.dma_start(out=wv_sb[:, kc, :], in_=wv_ch[:, kc, :])
                for cb in range(1, 4):
                    nc.sync.dma_start(
                        out=xv_cb[cb], in_=xv_ch[:, :, cb * 512:(cb + 1) * 512]
                    )
                for sc in range(kc_lim):
                    cb, scl = sc // 4, sc % 4
                    ps = pvp.tile([P, HDH], F32, tag="pv")
                    for kc in range(DC):
                        lhsT = xv_cb[cb][:, kc, scl * P:(scl + 1) * P]
                        for nh in range(2):
                            nc.tensor.matmul(
                                ps[:, nh * 512:(nh + 1) * 512],
                                lhsT,
                                wv_sb[:, kc, nh * 512:(nh + 1) * 512],
                                start=(kc == 0),
                                stop=(kc == DC - 1),
                            )
                    # strided copy into the interleaved layout
                    nc.vector.tensor_copy(
                        v_sb[:, sc, :, 0:DH],
                        ps.rearrange("p (h d) -> p h d", d=DH),
                    )
                    nc.vector.tensor_scalar(
                        v_sb[:, sc, :, DH:VW], v_sb[:, sc, :, 0:1],
                        0.0, 1.0, AOp.mult, AOp.add,
                    )

            # ---- phase K: KT -> DRAM, all pairs --------------------------
            with (
                tc.tile_pool(name="xk", bufs=1) as xkp,
                tc.tile_pool(name="wkp", bufs=1) as wkp,
                tc.tile_pool(name="ktst", bufs=3) as ktstp,
                tc.tile_pool(name="pk", bufs=2, space="PSUM") as pkp,
            ):
                xk_cb = []
                xk_ch = xkT.rearrange("(c p) s -> p c s", p=P)
                for cb in range(4):
                    t = xkp.tile([P, DC, 512], F32R, tag=f"xk{cb}",
                                 name=f"xk{cb}")
                    eng = nc.sync if cb % 2 == 0 else nc.scalar
                    eng.dma_start(
                        out=t, in_=xk_ch[:, :, cb * 512:(cb + 1) * 512]
                    )
                    xk_cb.append(t)
                wk_sb = wkp.tile([P, DC, HDH], F32R)
                wk_ch = wk.rearrange("(c p) n -> p c n", p=P)
                for kc in range(DC):
                    eng = nc.scalar if kc % 2 == 0 else nc.sync
                    eng.dma_start(out=wk_sb[:, kc, :], in_=wk_ch[:, kc, :])
                n_kb = kpad // 512   # 512-col blocks of KT kept
                for half in range(2):
                    blks = [b for b in (2 * half, 2 * half + 1) if b < n_kb]
                    if not blks:
                        continue
                    w_cols = 512 * len(blks)
                    for j in range(NJ):
                        ps = pkp.tile([P, SQ], F32, tag="pk")
                        for kc in range(DC):
                            lhsT = wk_sb[:, kc, j * P:(j + 1) * P]
                            for bi, b in enumerate(blks):
                                nc.tensor.matmul(
                                    ps[:, bi * 512:(bi + 1) * 512],
                                    lhsT,
                                    xk_cb[b][:, kc, :],
                                    start=(kc == 0),
                                    stop=(kc == DC - 1),
                                )
                        st = ktstp.tile([P, SQ], F32R, tag="ktst")
                        nc.vector.tensor_copy(st[:, 0:w_cols], ps[:, 0:w_cols])
                        nc.gpsimd.dma_start(
                            out=kt_dram[j][:, half * SQ:half * SQ + w_cols],
                            in_=st[:, 0:w_cols],
                        )

            # ---- phase Q: QT resident, all pairs -------------------------
            with tc.tile_pool(name="qtres", bufs=1) as qtpool:
                qt = [
                    qtpool.tile([P, SQ], F32R, tag=f"qt{j}", name=f"qt{j}")
                    for j in range(NJ)
                ]
                with (
                    tc.tile_pool(name="xq", bufs=1) as xqp,
                    tc.tile_pool(name="wqp", bufs=1) as wqp,
                    tc.tile_pool(name="pq", bufs=2, space="PSUM") as pqp,
                ):
                    xq_cb = []
                    xq_ch = xqT.rearrange("(c p) s -> p c s", p=P)
                    for cb in range(2):
                        t = xqp.tile([P, DC, 512], F32R, tag=f"xq{cb}",
                                     name=f"xq{cb}")
                        eng = nc.sync if cb % 2 == 0 else nc.scalar
                        eng.dma_start(
                            out=t, in_=xq_ch[:, :, cb * 512:(cb + 1) * 512]
                        )
                        xq_cb.append(t)
                    wq_sb = wqp.tile([P, DC, HDH], F32R)
                    wq_ch = wq.rearrange("(c p) n -> p c n", p=P)
                    for kc in range(DC):
                        eng = nc.scalar if kc % 2 == 0 else nc.sync
                        eng.dma_start(out=wq_sb[:, kc, :], in_=wq_ch[:, kc, :])
                    for j in range(NJ):
                        ps = pqp.tile([P, SQ], F32, tag="pq")
                        for kc in range(DC):
                            lhsT = wq_sb[:, kc, j * P:(j + 1) * P]
                            for nh in range(2):
                                nc.tensor.matmul(
                                    ps[:, nh * 512:(nh + 1) * 512],
                                    lhsT,
                                    xq_cb[nh][:, kc, :],
                                    start=(kc == 0),
                                    stop=(kc == DC - 1),
                                )
                        nc.vector.tensor_scalar(
                            qt[j], ps, 0.125, bq8_sb[:, j:j + 1],
                            AOp.mult, AOp.add,
                        )

                # ---- attention -------------------------------------------
                with (
                    tc.tile_pool(name="otres", bufs=1) as otpool,
                    tc.tile_pool(name="wopre", bufs=1) as wopre,
                ):
                    wo_pre = wopre.tile([P, 16, P], F32R)
                    for jj in range(NJ):
                        nc.sync.dma_start(
                            out=wo_pre[:, jj, :],
                            in_=wo[jj * P:(jj + 1) * P, 0:P],
                        )
                        nc.sync.dma_start(
                            out=wo_pre[:, NJ + jj, :],
                            in_=wo[jj * P:(jj + 1) * P, P:2 * P],
                        )
                    ot = [
                        otpool.tile([P, SQ], F32R, tag=f"ot{j}", name=f"ot{j}")
                        for j in range(NJ)
                    ]
                    with (
                        tc.tile_pool(name="ktsb", bufs=2) as ktp,
                        tc.tile_pool(name="expp", bufs=3) as expp,
                        tc.tile_pool(name="lbp", bufs=2) as lbp,
                        tc.tile_pool(name="ps_s", bufs=1, space="PSUM") as pss,
                        tc.tile_pool(name="ps_o", bufs=1, space="PSUM") as pso,
                    ):
                        kt_tiles = {}

                        def load_kt(jj):
                            t = ktp.tile([P, kpad], F32R, tag="kt", name="kt_sb")
                            nc.sync.dma_start(out=t, in_=kt_dram[jj][:])
                            kt_tiles[jj] = t

                        load_kt(0)
                        for j in range(NJ):
                            if j + 1 < NJ:
                                load_kt(j + 1)  # prefetch next pair's KT
                            kt_sb = kt_tiles.pop(j)
                            ps_oa = pso.tile([VW, SQ], F32, tag="oa")
                            ps_ob = pso.tile([VW, SQ], F32, tag="ob")
                            ets = {}

                            def scores_exp(kc, j=j, kt_sb=kt_sb, ets=ets):
                                ps_s = pss.tile(
                                    [P, 2 * SQ], F32, tag="s", name="ps_s"
                                )
                                for hh in (0, 1):
                                    lhsT = kt_sb[hh * 64:(hh + 1) * 64,
                                                 kc * P:(kc + 1) * P]
                                    for nh in range(2):
                                        nc.tensor.matmul(
                                            ps_s[:, hh * SQ + nh * 512:
                                                 hh * SQ + (nh + 1) * 512],
                                            lhsT,
                                            qt[j][hh * 64:(hh + 1) * 64,
                                                  nh * 512:(nh + 1) * 512],
                                            tile_position=(hh * 64, 0),
                                        )
                                et = expp.tile(
                                    [P, 2 * SQ], F32R, tag="e", name="et"
                                )
                                nc.scalar.activation(
                                    et, ps_s, Exp,
                                    bias=maskb_sb[:, kc:kc + 1], scale=1.0,
                                )
                                ets[kc] = et

                            def pv(kc, j=j, ets=ets, ps_oa=ps_oa, ps_ob=ps_ob):
                                et = ets.pop(kc)
                                for hh, ps_o in ((0, ps_oa), (1, ps_ob)):
                                    vh = v_sb[:, kc, 2 * j + hh, :]  # [128,65]
                                    for nh in range(2):
                                        nc.tensor.matmul(
                                            ps_o[:, nh * 512:(nh + 1) * 512],
                                            vh,
                                            et[:, hh * SQ + nh * 512:
                                               hh * SQ + (nh + 1) * 512],
                                            start=(kc == 0),
                                            stop=(kc == kc_lim - 1),
                                        )

                            scores_exp(0)
                            scores_exp(1)
                            for kc in range(2, kc_lim):
                                scores_exp(kc)
                                pv(kc - 2)
                            pv(kc_lim - 2)
                            pv(kc_lim - 1)

                            # release PSUM fast: copy both heads to SBUF
                            # (incl. l rows); broadcast l, recip, then scale
                            cpA = lbp.tile([VW, SQ], F32, tag="cpA", bufs=2)
                            nc.vector.tensor_copy(cpA, ps_oa)
                            cpB = lbp.tile([VW, SQ], F32R, tag="cpB", bufs=2)
                            nc.vector.tensor_copy(cpB, ps_ob)
                            L_sb = lbp.tile([P, SQ], F32, tag="L", bufs=2)
                            for hh, rsrc in ((0, cpA), (1, cpB)):
                                rd = rdp.tile(
                                    [1, SQ], F32, tag="rd", name="rd"
                                )
                                nc.sync.dma_start(
                                    out=rd, in_=rsrc[DH:VW, :].bitcast(F32)
                                )
                                rd_b = bass.AP(
                                    tensor=rd.tensor, offset=rd.offset,
                                    ap=[[0, 64], rd.ap[-1]],
                                )
                                nc.sync.dma_start(
                                    out=L_sb[hh * 64:(hh + 1) * 64, :],
                                    in_=rd_b,
                                )
                            nc.vector.reciprocal_approx_fast(L_sb, L_sb)
                            nc.vector.tensor_mul(
                                ot[j][0:64, :], cpA[0:DH, :], L_sb[0:64, :]
                            )
                            nc.gpsimd.dma_start(
                                out=ot[j][64:128, :], in_=cpB[0:DH, :]
                            )
                            nc.vector.tensor_mul(
                                ot[j][64:128, :], ot[j][64:128, :],
                                L_sb[64:128, :],
                            )

                    # ---- output projection -------------------------------
                    with (
                        tc.tile_pool(name="wop", bufs=8) as wop,
                        tc.tile_pool(name="ytp", bufs=3) as ytp,
                        tc.tile_pool(name="py", bufs=2, space="PSUM") as pyp,
                    ):
                        yt_ch = yT.rearrange("(c p) s -> c p s", p=P)
                        for dc in range(DC):
                            ps = pyp.tile([P, SQ], F32, tag="py")
                            for j in range(NJ):
                                if dc < 2:
                                    wo_t = wo_pre[:, dc * NJ + j, :]
                                else:
                                    wo_t = wop.tile([P, P], F32R, tag="wo")
                                    nc.scalar.dma_start(
                                        out=wo_t,
                                        in_=wo[j * P:(j + 1) * P,
                                               dc * P:(dc + 1) * P],
                                    )
                                for nh in range(2):
                                    nc.tensor.matmul(
                                        ps[:, nh * 512:(nh + 1) * 512],
                                        wo_t,
                                        ot[j][:, nh * 512:(nh + 1) * 512],
                                        start=(j == 0),
                                        stop=(j == NJ - 1),
                                    )
                            yt_sb = ytp.tile([P, SQ], F32, tag="yt")
                            nc.vector.tensor_scalar(
                                yt_sb, ps, bo2_sb[:, dc:dc + 1], None, AOp.add
                            )
                            nc.gpsimd.dma_start(out=yt_ch[dc], in_=yt_sb)

    nc.compile()
    return nc


def _prepare(x_Q, x_K, x_V, src_batch_lens, Wq, bq, Wk, bk, Wv, bv, Wo, bo):
    x_Q = np.asarray(x_Q, dtype=np.float32)
    x_K = np.asarray(x_K, dtype=np.float32)
    x_V = np.asarray(x_V, dtype=np.float32)
    lens = np.asarray(src_batch_lens)
    Wq = np.ascontiguousarray(np.asarray(Wq, dtype=np.float32))
    Wk = np.ascontiguousarray(np.asarray(Wk, dtype=np.float32))
    Wv = np.ascontiguousarray(np.asarray(Wv, dtype=np.float32))
    Wo = np.ascontiguousarray(np.asarray(Wo, dtype=np.float32))
    bq = np.asarray(bq, dtype=np.float32)
    bv = np.asarray(bv, dtype=np.float32)
    bo = np.asarray(bo, dtype=np.float32)

    maxlen = int(np.max(lens))
    maxlen = max(1, min(S, maxlen))
    kc_lim = (maxlen + P - 1) // P

    bo2_full = (bv @ Wo + bo).astype(np.float32)
    bo2 = np.ascontiguousarray(bo2_full.reshape(DC, P).T)
    bq8 = np.ascontiguousarray((bq / 8.0).reshape(NJ, P).T)

    in_maps = []
    for c in range(8):
        b, hh = c // 2, c % 2
        q0 = hh * SQ
        k_idx = np.arange(S)
        mvec = np.where(k_idx < int(lens[b]), 0.0, MASK_NEG).astype(np.float32)
        in_maps.append({
            "xqT": np.ascontiguousarray(x_Q[b, q0:q0 + SQ, :].T),
            "xkT": np.ascontiguousarray(x_K[b].T),
            "xvT": np.ascontiguousarray(x_V[b].T),
            "wq": Wq, "wk": Wk, "wv": Wv, "wo": Wo,
            "bq8": bq8, "bo2": bo2,
            "maskb": np.ascontiguousarray(mvec.reshape(KC, P).T),
        })
    return kc_lim, in_maps


def _build_in_maps(inputs):
    return _prepare(**inputs)[1]


def kernel(x_Q, x_K, x_V, src_batch_lens, Wq, bq, Wk, bk, Wv, bv, Wo, bo):
    kc_lim, in_maps = _prepare(x_Q, x_K, x_V, src_batch_lens,
                               Wq, bq, Wk, bk, Wv, bv, Wo, bo)
    if kc_lim not in _CACHE:
        _CACHE[kc_lim] = build_bass(kc_lim)
    nc = _CACHE[kc_lim]

    res = run_bass_kernel_spmd(nc, in_maps, core_ids=list(range(8)))

    out = np.empty((B, S, D), dtype=np.float32)
    for c in range(8):
        b, hh = c // 2, c % 2
        q0 = hh * SQ
        out[b, q0:q0 + SQ, :] = res.results[c]["yT"].T
    return out
